# revision 48
# baseline (speedup 1.0000x reference)
"""AWLoss (adaptive Wiener filter loss) Trainium2 kernel, 8-core data-parallel.

Math (analytic reduction verified against the reference to ~2e-8 rel err):
  The penalty T (std=1e-4) is numerically 1 everywhere except 0 at the center
  pixel, and the roll puts that center at pre-roll [0,0]. The loss collapses to
      loss = sum_{b,c} 0.5 * (1 - v00^2 / E)
  with, per (b,c) pair (x = target, y = recon, A = 511x256 padded-DFT matrix):
      Fx = A x A^T,  Fy = A y A^T          (full 511x511 spectra)
      a = |Fx|^2, cr = Re(conj(Fx) Fy), b = |Fy|^2
      v00 = sum(cr/a) / N^2,   E = sum(b/a) / N^2       (N = 511)
  The flip-phase factor cancels between numerator and denominator and the
  eps=1e-9 pre-whitening is negligible (|Fx|^2 >~ 0.5 everywhere).

Spectral coverage: the device computes q-sums over rows k = 0..255 ("+"
fields) and rows (511-k) mod 511 ("-" fields, via conjugated DFT stacks
re-using the same Ut), columns l = 0..255. With Hermitian symmetry
  S_full = 2*(S_device - S_row0_half) - S_col0
where the k=0 row (duplicated on device) and l=0 column sums are recomputed
exactly on the host from row/column sums of x and y (tiny 1-D DFTs).

Device pipeline, per (b,c) pair, software-pipelined over 12 (pair, l-chunk)
chunks (all matmuls fp8-e4m3 with DoubleRow pairing, fp32 PSUM accum):
  step1 (PE):  Ut[n,k] = sum_m x[m,n] A[k,m], k = 0..255
  ut copy (DVE): PSUM -> SBUF fp8
  step2 (PE):  F(+-)[l,k] = sum_n B(+-)[n,l] Ut[n,k], per l-chunk of 128
  cx/cy (ACT): F fields PSUM -> SBUF bf16
  sq,ry (POOL), a (DVE): |Fx|^2 and ry = Fy * rw
  rw (DVE):    1/a via the bf16 bit-hack (0x7EF0 - bits, ~3% err; the loss
               term tolerates ~100x more)
  dots (PE):   S1 = sum cx.ry, S2 = sum cy.ry as Gram-matrix accumulations
               over 128x128 chunks; diagonal extracted with a
               scalar_tensor_tensor against the identity (accum_out).
Host: Hermitian corrections, v00/E ratios, final sum in float64.

Toolchain notes: bacc.Bacc + explicit finalize() (walrus allows at most one
sem wait per instruction; Bacc's generate_event_semaphores splits them).
Engine-legality notes (BIR verifier): GPSIMD/Pool cannot touch PSUM (all
PSUM drains must be ACT or DVE), and dma_start cannot source PSUM -- both
constraints shape the drain/dot structure above. Lead-in: pair-0's step1-y
borrows the empty fy PSUM buffer so it needn't WAR-wait on the utx drain
(the fy ring then orders the first fy matmuls after the uty drain).
"""

import os
import sys
from contextlib import ExitStack
from functools import lru_cache

import numpy as np

sys.path.insert(0, "/opt/trn_rl_repo")

import ml_dtypes

import concourse.bacc as bacc
import concourse.mybir as mybir
import concourse.tile as tile
from concourse.bass_utils import run_bass_kernel_spmd

dt = mybir.dt
ALU = mybir.AluOpType

N = 511
OFF = 127
P = 256
NCORES = 8
PAIRS_PER_CORE = 6  # 48 (b,c) pairs / 8 cores


# ---------------------------------------------------------------- constants
@lru_cache(maxsize=1)
def _consts():
    k = np.arange(256)
    m = np.arange(P)
    # step1 rhs AT[m, k] = A[k, m] = exp(-2pi i k (m+OFF) / N), k = 0..255
    theta = 2.0 * np.pi * np.outer(k, (m + OFF)) / N  # [k, m]
    atc = np.stack([np.cos(theta).T, -np.sin(theta).T])  # [2, m, k]
    atc = atc.reshape(2, 128, 2, 256)  # m -> (i, j) DoubleRow pairs
    # step2 lhsT B[n, l] = A[l, n]: Br = cos(2pi l (n+OFF)/N), Bi = -sin
    phi = 2.0 * np.pi * np.outer(m + OFF, np.arange(256)) / N  # [n, l]
    Br = np.cos(phi)
    Bi = -np.sin(phi)
    # stacked lhsT over contraction rows s: s=0,1 multiply Utr, s=2,3 Uti.
    # F+ = (Br + iBi).(Utr + iUti):  F+r = Br.Utr - Bi.Uti ; F+i = Bi.Utr + Br.Uti
    # F- = (Br + iBi).conj(Ut):      F-r = Br.Utr + Bi.Uti ; F-i = Bi.Utr - Br.Uti
    b2 = np.empty((2, 2, 2, 256, 256))  # [pm, oc, comp(of Ut), n, l]
    for pm, sgn in ((0, 1.0), (1, -1.0)):
        b2[pm, 0, 0] = Br
        b2[pm, 1, 0] = Bi
        b2[pm, 0, 1] = -sgn * Bi
        b2[pm, 1, 1] = sgn * Br
    # DoubleRow pairing n = 2i + j: [pm, oc, comp, i, j, l] -> [i, lt, (pm oc), comp, j, l%128]
    # flattened per partition so the load DMA is one contiguous 4 KB run
    # (>=512 B inner run avoids the cost model's 2x small-elem latency mult).
    b2 = b2.reshape(2, 2, 2, 128, 2, 2, 128)  # [pm, oc, comp, i, j, lt, l128]
    bstk2 = np.ascontiguousarray(
        np.transpose(b2, (3, 5, 0, 1, 2, 4, 6)).reshape(128, 4096)
    )
    bf16 = ml_dtypes.bfloat16
    f8 = mybir.dt.np(dt.float8e4)
    # full complex A (float64) for host-side row/col corrections
    kk = np.arange(N)
    A_full = np.exp(-2j * np.pi * np.outer(kk, (m + OFF)) / N)  # [511, 256]
    ident = np.eye(128)
    return atc.astype(f8), bstk2.astype(f8), ident.astype(bf16), A_full


# ---------------------------------------------------------------- bass build
_NC = None


def _build():
    global _NC
    if _NC is not None:
        return _NC
    nc = bacc.Bacc()
    xy_d = nc.dram_tensor("xy", [PAIRS_PER_CORE, 128, 2, 2, P], dt.float8e4, kind="ExternalInput")
    atc_d = nc.dram_tensor("atc", [2, 128, 2, 256], dt.float8e4, kind="ExternalInput")
    bstk_d = nc.dram_tensor("bstk", [128, 4096], dt.float8e4, kind="ExternalInput")
    out_d = nc.dram_tensor("out", [PAIRS_PER_CORE, 128, 2, 128], dt.float32, kind="ExternalOutput")

    with tile.TileContext(nc) as tc:
        with ExitStack() as ctx:
            consts = ctx.enter_context(tc.tile_pool(name="consts", bufs=1))
            # ACT table prime: pay the 1283 ns activation-table load at t~0
            # on the idle ACT engine instead of inside the first real drain.
            prime = consts.tile([128, 1], dt.bfloat16, tag="prime")
            prime2 = consts.tile([128, 1], dt.bfloat16, tag="prime2")
            nc.gpsimd.memset(prime, 0.0)
            nc.scalar.copy(prime2, prime)
            # AT tiles (fp8 DoubleRow): [p=i, comp, j, k] on the ACT hwdge
            # queue so it doesn't serialize behind the pair-0 input DMA.
            at_sb = consts.tile([128, 2, 2, 256], dt.float8e4)
            nc.scalar.dma_start(at_sb, atc_d.rearrange("c p j k -> p c j k"))
            # B tiles (fp8 DoubleRow): [p=i, lt, g=(pm,oc), comp, j, l%128],
            # shipped pre-flattened so the load is ONE contiguous 4 KB/row DMA
            # (1x latency class) on the gpsimd queue.
            b_sb = consts.tile([128, 2, 4, 2, 2, 128], dt.float8e4)
            nc.gpsimd.dma_start(b_sb, bstk_d[:])
            # inputs (fp8 DoubleRow): per-pair tiles [p=i, which, j, n], m = 2i+j
            # one combined x+y DMA per pair (contiguous 1 KB rows)
            xy_tiles = []
            for q in range(PAIRS_PER_CORE):
                xyq = consts.tile([128, 2, 2, P], dt.float8e4, tag=f"xy{q}")
                nc.sync.dma_start(xyq, xy_d[q])
                xy_tiles.append(xyq)

            utps_pool = ctx.enter_context(tc.tile_pool(name="utps", bufs=1, space="PSUM"))
            fps_pool = ctx.enter_context(tc.tile_pool(name="fps", bufs=1, space="PSUM"))
            dot_pool = ctx.enter_context(tc.tile_pool(name="dot", bufs=2, space="PSUM"))
            utsb_pool = ctx.enter_context(tc.tile_pool(name="utsb", bufs=8))
            elem_pool = ctx.enter_context(tc.tile_pool(name="elem", bufs=12))

            def step1(p, w_i):
                """DFT step 1 for pair p, input w_i (0=x,1=y): Ut[n, k0..255]."""
                if p == 0 and w_i == 1:
                    # lead-in: borrow the (still empty) fy PSUM buffer so
                    # step1-y doesn't WAR-wait on the utx drain; the fy ring
                    # naturally orders the first fy matmuls after this drain.
                    ut_ps = fps_pool.tile([128, 2, 2, 256], dt.float32, tag="fy")
                else:
                    ut_ps = utps_pool.tile([128, 2, 2, 256], dt.float32, tag="utps")
                for eo in range(2):
                    lhsT = xy_tiles[p][:, w_i, :, eo:P:2]
                    for comp in range(2):
                        nc.tensor.matmul(
                            ut_ps[:, comp, eo, :],
                            lhsT,
                            at_sb[:, comp],
                            start=True,
                            stop=True,
                            perf_mode=mybir.MatmulPerfMode.DoubleRow,
                        )
                # PSUM -> SBUF fp8; [p=n%128, comp, even/odd, k]
                ut_sb = utsb_pool.tile([128, 2, 2, 256], dt.float8e4, tag=f"utsb{w_i}")
                if p == 0 and w_i == 0:
                    # warmup: split the first drain ACT||DVE so step2 (and the
                    # whole elementwise pipeline) primes ~500 ns earlier
                    nc.scalar.copy(ut_sb[:, 0], ut_ps[:, 0])
                    nc.vector.tensor_copy(ut_sb[:, 1], ut_ps[:, 1])
                else:
                    nc.vector.tensor_copy(ut_sb, ut_ps)
                return ut_sb

            def step2_lt(utx, uty, lt):
                """DFT step 2 for l-chunk lt: fields [l%128, w, pm, oc, k] in PSUM."""
                fx_ps = fps_pool.tile([128, 2, 2, 256], dt.float32, tag="fx")
                fy_ps = fps_pool.tile([128, 2, 2, 256], dt.float32, tag="fy")
                for pm in range(2):
                    for oc in range(2):
                        for comp in range(2):
                            lhsT = b_sb[:, lt, pm * 2 + oc, comp]
                            nc.tensor.matmul(
                                fx_ps[:, pm, oc, :], lhsT, utx[:, comp],
                                start=(comp == 0), stop=(comp == 1),
                                perf_mode=mybir.MatmulPerfMode.DoubleRow,
                            )
                            nc.tensor.matmul(
                                fy_ps[:, pm, oc, :], lhsT, uty[:, comp],
                                start=(comp == 0), stop=(comp == 1),
                                perf_mode=mybir.MatmulPerfMode.DoubleRow,
                            )
                return fx_ps, fy_ps

            def elem_a(p, lt, fx_ps, fy_ps):
                # stage A: drain PSUM -> SBUF bf16, square, |Fx|^2
                cx = elem_pool.tile([128, 2, 2, 256], dt.bfloat16, tag="cx")
                nc.scalar.copy(cx, fx_ps)
                cy = elem_pool.tile([128, 2, 2, 256], dt.bfloat16, tag="cy")
                nc.scalar.copy(cy, fy_ps)
                sq = elem_pool.tile([128, 2, 2, 256], dt.bfloat16, tag="sq")
                if p == PAIRS_PER_CORE - 1 and lt == 1:
                    # drain phase: keep the serial sq->a->rw chain on DVE to
                    # skip two cross-engine semaphore hops
                    nc.vector.tensor_mul(sq, cx, cx)
                else:
                    nc.gpsimd.tensor_mul(sq, cx, cx)
                a_sb = elem_pool.tile([128, 2, 256], dt.bfloat16, tag="a")
                nc.vector.tensor_add(a_sb, sq[:, :, 0], sq[:, :, 1])
                return cx, cy, a_sb

            def elem_b(state):
                # stage B: rw ~= 1/a via the bf16 bit-hack (magic - bits,
                # ~3% rel err -- the loss tolerates ~100x more), ry = Fy * rw
                p, lt, cx, cy, a_sb = state
                rw = elem_pool.tile([128, 2, 256], dt.bfloat16, tag="rw")
                nc.vector.tensor_scalar(
                    rw.bitcast(dt.uint16), a_sb.bitcast(dt.uint16),
                    -1.0, float(0x7EF0), op0=ALU.mult, op1=ALU.add,
                )
                ry = elem_pool.tile([128, 2, 2, 256], dt.bfloat16, tag="ry")
                if p == PAIRS_PER_CORE - 1 and lt == 1:
                    # drain phase: DVE is idle here and the two POOL TTs would
                    # serialize on the tail critical path
                    nc.vector.tensor_mul(ry[:, :, 0], cy[:, :, 0], rw)
                    nc.gpsimd.tensor_mul(ry[:, :, 1], cy[:, :, 1], rw)
                else:
                    nc.gpsimd.tensor_mul(ry[:, :, 0], cy[:, :, 0], rw)
                    nc.gpsimd.tensor_mul(ry[:, :, 1], cy[:, :, 1], rw)
                return p, lt, cx, cy, ry

            pair_d = {}

            def elem_c(state):
                # stage C: Gram-accumulate dots for this chunk; both l-chunks
                # of a pair share one PSUM bank, extracted once per pair.
                # Tail shortcut: the LAST pair stops its Gram after l-chunk 0
                # (extraction overlaps chunk 1) and computes chunk 1's dots as
                # two direct STT reductions -- no PSUM Gram / extraction on
                # the critical tail path.
                p, lt, cx, cy, ry = state
                c0 = p * 8
                _elem_c_body(p, lt, cx, cy, ry, c0)

            def _elem_c_body(p, lt, cx, cy, ry, c0):
                if lt == 0:
                    d_tile = dot_pool.tile([128, 2, 128], dt.float32, tag="d")
                    pair_d[p] = d_tile
                d = pair_d[p]
                # S1 group first (oc=0 blocks lead: their ry half lands first).
                order = (0, 1, 4, 5, 2, 3, 6, 7)
                for which in range(2):
                    op = cx if which == 0 else cy
                    for i, c in enumerate(order):
                        a_i, b_i, h = c // 4, (c // 2) % 2, c % 2
                        opc = op[:, a_i, b_i, h * 128 : (h + 1) * 128]
                        ryc = ry[:, a_i, b_i, h * 128 : (h + 1) * 128]
                        st = (lt == 0 and i == 0)
                        sp = (lt == 1 and i == 7)
                        nc.tensor.matmul(d[:, which], opc, ryc, start=st, stop=sp)
                if lt == 1:
                    # ship the raw Gram; the host extracts the two diagonals.
                    # One 256-free copy replaces the two diag-STTs (516 ns),
                    # and the final DMA slice overlaps nothing downstream.
                    dsb = elem_pool.tile([128, 2, 128], dt.float32, tag="dsb")
                    if p == PAIRS_PER_CORE - 1:
                        # tail: ACT is idle after the last drain
                        nc.scalar.copy(dsb, d)
                    else:
                        nc.vector.tensor_copy(dsb, d)
                    # alternate DMA queues so two pairs' outputs never
                    # serialize on one queue in the tail
                    (nc.sync if p % 2 == 0 else nc.scalar).dma_start(out_d[p], dsb)
                    del pair_d[p]

            def chunk_a(p, utx, uty, lt):
                fx_ps, fy_ps = step2_lt(utx, uty, lt)
                return (p, lt) + elem_a(p, lt, fx_ps, fy_ps)

            # 3-deep software pipeline over the 12 (pair, lt) chunks:
            # A(c) | B(c-1) | C(c-2), with step1 of the next pair interleaved.
            # advance takes a THUNK so the older chunks' dots/extractions are
            # emitted (and queued) BEFORE the new chunk's step2/drains --
            # frees dot PSUM banks earlier and keeps the tail DVE chain clean.
            stage_b = stage_c = None

            def advance(state_a):
                nonlocal stage_b, stage_c
                if stage_c is not None:
                    elem_c(stage_c)
                    stage_c = None
                if stage_b is not None:
                    stage_c = elem_b(stage_b)
                stage_b = state_a

            pending = None
            for p in range(PAIRS_PER_CORE):
                utx = step1(p, 0)
                if pending is not None:
                    advance(chunk_a(pending[0], pending[1], pending[2], 0))
                    advance(chunk_a(pending[0], pending[1], pending[2], 1))
                uty = step1(p, 1)
                pending = (p, utx, uty)
            q, ux, uy = pending
            advance(chunk_a(q, ux, uy, 0))
            advance(chunk_a(q, ux, uy, 1))
            advance(None)
            if stage_c is not None:
                elem_c(stage_c)

    nc.finalize()  # Bacc: runs wait-splitting (1-wait/inst HW limit), reg alloc
    _NC = nc
    return nc


# ---------------------------------------------------------------- host side
def _host_corrections(x, y, A_full):
    """Exact (float64) k=0-row and l=0-col sums of cr/a and b/a for one pair."""
    x = x.astype(np.float64)
    y = y.astype(np.float64)
    # l=0 column: F[k,0] = A @ row-sums (sum over n)
    Fx0 = A_full @ x.sum(axis=1)
    Fy0 = A_full @ y.sum(axis=1)
    a0 = np.abs(Fx0) ** 2
    s1c = ((np.conj(Fx0) * Fy0).real / a0).sum()
    s2c = (np.abs(Fy0) ** 2 / a0).sum()
    # k=0 row, l=0..255: F[0,l] = A[:256] @ col-sums (sum over m)
    Fx1 = A_full[:256] @ x.sum(axis=0)
    Fy1 = A_full[:256] @ y.sum(axis=0)
    a1 = np.abs(Fx1) ** 2
    s1r = ((np.conj(Fx1) * Fy1).real / a1).sum()
    s2r = (np.abs(Fy1) ** 2 / a1).sum()
    return s1c, s2c, s1r, s2r


def kernel(recon, target):
    atc, bstk, ident, A_full = _consts()
    f8 = mybir.dt.np(dt.float8e4)
    xs = target.reshape(48, 128, 2, P).astype(f8)  # x = target; m -> (i, j)
    ys = recon.reshape(48, 128, 2, P).astype(f8)  # y = recon
    xy = np.ascontiguousarray(np.stack([xs, ys], axis=2))  # [48, 128, 2, 2, 256]

    nc = _build()
    in_maps = [
        {
            "xy": xy[c * PAIRS_PER_CORE : (c + 1) * PAIRS_PER_CORE],
            "atc": atc,
            "bstk": bstk,
        }
        for c in range(NCORES)
    ]
    res = None
    for attempt in range(3):
        try:
            res = run_bass_kernel_spmd(nc, in_maps, core_ids=list(range(NCORES)))
            break
        except Exception:
            if attempt == 2:
                raise
            import time as _time

            _time.sleep(2.0)

    NN = float(N) * float(N)
    loss = 0.0
    for c in range(NCORES):
        grams = res.results[c]["out"].astype(np.float64)  # [6, 128, 2, 128]
        kk = np.arange(128)
        for p in range(PAIRS_PER_CORE):
            s1_dev = grams[p, kk, 0, kk]
            s2_dev = grams[p, kk, 1, kk]
            pair = c * PAIRS_PER_CORE + p
            b, ch = divmod(pair, 3)
            s1c, s2c, s1r, s2r = _host_corrections(
                np.asarray(target[b, ch]), np.asarray(recon[b, ch]), A_full
            )
            S1 = 2.0 * (s1_dev.sum() - s1r) - s1c
            S2 = 2.0 * (s2_dev.sum() - s2r) - s2c
            v00 = S1 / NN
            E = S2 / NN
            loss += 0.5 * (1.0 - v00 * v00 / E)
    return np.float32(loss)



# revision 59
# speedup vs baseline: 1.0344x; 1.0344x over previous
"""AWLoss (adaptive Wiener filter loss) Trainium2 kernel, 8-core data-parallel.

Math (analytic reduction verified against the reference to ~2e-8 rel err):
  The penalty T (std=1e-4) is numerically 1 everywhere except 0 at the center
  pixel, and the roll puts that center at pre-roll [0,0]. The loss collapses to
      loss = sum_{b,c} 0.5 * (1 - v00^2 / E)
  with, per (b,c) pair (x = target, y = recon, A = 511x256 padded-DFT matrix):
      Fx = A x A^T,  Fy = A y A^T          (full 511x511 spectra)
      a = |Fx|^2, cr = Re(conj(Fx) Fy), b = |Fy|^2
      v00 = sum(cr/a) / N^2,   E = sum(b/a) / N^2       (N = 511)
  The flip-phase factor cancels between numerator and denominator and the
  eps=1e-9 pre-whitening is negligible (|Fx|^2 >~ 0.5 everywhere).

Spectral coverage: the device computes q-sums over rows k = 0..255 ("+"
fields) and rows (511-k) mod 511 ("-" fields, via conjugated DFT stacks
re-using the same Ut), columns l = 0..255. With Hermitian symmetry
  S_full = 2*(S_device - S_row0_half) - S_col0
where the k=0 row (duplicated on device) and l=0 column sums are recomputed
exactly on the host from row/column sums of x and y (tiny 1-D DFTs).

Device pipeline, per (b,c) pair, software-pipelined over 12 (pair, l-chunk)
chunks (all matmuls fp8-e4m3 with DoubleRow pairing, fp32 PSUM accum):
  step1 (PE):  Ut[n,k] = sum_m x[m,n] A[k,m], k = 0..255
  ut copy (DVE): PSUM -> SBUF fp8
  step2 (PE):  F(+-)[l,k] = sum_n B(+-)[n,l] Ut[n,k], per l-chunk of 128
  cx/cy (ACT): F fields PSUM -> SBUF bf16
  sq,ry (POOL), a (DVE): |Fx|^2 and ry = Fy * rw
  rw (DVE):    1/a via the bf16 bit-hack (0x7EF0 - bits, ~3% err; the loss
               term tolerates ~100x more)
  dots (PE):   S1 = sum cx.ry, S2 = sum cy.ry as Gram-matrix accumulations
               over 128x128 chunks; diagonal extracted with a
               scalar_tensor_tensor against the identity (accum_out).
Host: Hermitian corrections, v00/E ratios, final sum in float64.

Toolchain notes: bacc.Bacc + explicit finalize() (walrus allows at most one
sem wait per instruction; Bacc's generate_event_semaphores splits them).
Engine-legality notes (BIR verifier): GPSIMD/Pool cannot touch PSUM (all
PSUM drains must be ACT or DVE), and dma_start cannot source PSUM -- both
constraints shape the drain/dot structure above. Lead-in: pair-0's step1-y
borrows the empty fy PSUM buffer so it needn't WAR-wait on the utx drain
(the fy ring then orders the first fy matmuls after the uty drain).
"""

import os
import sys
from contextlib import ExitStack
from functools import lru_cache

import numpy as np

sys.path.insert(0, "/opt/trn_rl_repo")

import ml_dtypes

import concourse.bacc as bacc
import concourse.mybir as mybir
import concourse.tile as tile
from concourse.bass_utils import run_bass_kernel_spmd

dt = mybir.dt
ALU = mybir.AluOpType

N = 511
OFF = 127
P = 256
NCORES = 8
PAIRS_PER_CORE = 6  # 48 (b,c) pairs / 8 cores


# ---------------------------------------------------------------- constants
@lru_cache(maxsize=1)
def _consts():
    k = np.arange(256)
    m = np.arange(P)
    # step1 rhs AT[m, k] = A[k, m] = exp(-2pi i k (m+OFF) / N), k = 0..255
    theta = 2.0 * np.pi * np.outer(k, (m + OFF)) / N  # [k, m]
    atc = np.stack([np.cos(theta).T, -np.sin(theta).T])  # [2, m, k]
    atc = atc.reshape(2, 128, 2, 256)  # m -> (i, j) DoubleRow pairs
    # step2 lhsT B[n, l] = A[l, n]: Br = cos(2pi l (n+OFF)/N), Bi = -sin
    phi = 2.0 * np.pi * np.outer(m + OFF, np.arange(256)) / N  # [n, l]
    Br = np.cos(phi)
    Bi = -np.sin(phi)
    # stacked lhsT over contraction rows s: s=0,1 multiply Utr, s=2,3 Uti.
    # F+ = (Br + iBi).(Utr + iUti):  F+r = Br.Utr - Bi.Uti ; F+i = Bi.Utr + Br.Uti
    # F- = (Br + iBi).conj(Ut):      F-r = Br.Utr + Bi.Uti ; F-i = Bi.Utr - Br.Uti
    b2 = np.empty((2, 2, 2, 256, 256))  # [pm, oc, comp(of Ut), n, l]
    for pm, sgn in ((0, 1.0), (1, -1.0)):
        b2[pm, 0, 0] = Br
        b2[pm, 1, 0] = Bi
        b2[pm, 0, 1] = -sgn * Bi
        b2[pm, 1, 1] = sgn * Br
    # DoubleRow pairing n = 2i + j: [pm, oc, comp, i, j, l] -> [i, lt, (pm oc), comp, j, l%128]
    # flattened per partition so the load DMA is one contiguous 4 KB run
    # (>=512 B inner run avoids the cost model's 2x small-elem latency mult).
    b2 = b2.reshape(2, 2, 2, 128, 2, 2, 128)  # [pm, oc, comp, i, j, lt, l128]
    bstk2 = np.ascontiguousarray(
        np.transpose(b2, (3, 5, 0, 1, 2, 4, 6)).reshape(128, 4096)
    )
    bf16 = ml_dtypes.bfloat16
    f8 = mybir.dt.np(dt.float8e4)
    # full complex A (float64) for host-side row/col corrections
    kk = np.arange(N)
    A_full = np.exp(-2j * np.pi * np.outer(kk, (m + OFF)) / N)  # [511, 256]
    ident = np.eye(128)
    return atc.astype(f8), bstk2.astype(f8), ident.astype(bf16), A_full


# ---------------------------------------------------------------- bass build
_NC = None


def _build():
    global _NC
    if _NC is not None:
        return _NC
    nc = bacc.Bacc()
    xy_d = nc.dram_tensor("xy", [PAIRS_PER_CORE, 128, 2, 2, P], dt.float8e4, kind="ExternalInput")
    atc_d = nc.dram_tensor("atc", [2, 128, 2, 256], dt.float8e4, kind="ExternalInput")
    bstk_d = nc.dram_tensor("bstk", [128, 4096], dt.float8e4, kind="ExternalInput")
    out_d = nc.dram_tensor("out", [PAIRS_PER_CORE, 128, 2, 128], dt.float32, kind="ExternalOutput")
    # last pair's chunk-1 dot columns (tail shortcut: no Gram on the tail)
    out2_d = nc.dram_tensor("out2", [2, 128, 1], dt.float32, kind="ExternalOutput")

    with tile.TileContext(nc) as tc:
        with ExitStack() as ctx:
            consts = ctx.enter_context(tc.tile_pool(name="consts", bufs=1))
            # ACT table prime: pay the 1283 ns activation-table load at t~0
            # on the idle ACT engine instead of inside the first real drain.
            prime = consts.tile([128, 1], dt.bfloat16, tag="prime")
            prime2 = consts.tile([128, 1], dt.bfloat16, tag="prime2")
            nc.gpsimd.memset(prime, 0.0)
            nc.scalar.copy(prime2, prime)
            # AT tiles (fp8 DoubleRow): [p=i, comp, j, k] on the ACT hwdge
            # queue so it doesn't serialize behind the pair-0 input DMA.
            at_sb = consts.tile([128, 2, 2, 256], dt.float8e4)
            nc.scalar.dma_start(at_sb, atc_d.rearrange("c p j k -> p c j k"))
            # B tiles (fp8 DoubleRow): [p=i, lt, g=(pm,oc), comp, j, l%128],
            # shipped pre-flattened so the load is ONE contiguous 4 KB/row DMA
            # (1x latency class) on the gpsimd queue.
            b_sb = consts.tile([128, 2, 4, 2, 2, 128], dt.float8e4)
            nc.gpsimd.dma_start(b_sb, bstk_d[:])
            # inputs (fp8 DoubleRow): per-pair tiles [p=i, which, j, n], m = 2i+j
            # one combined x+y DMA per pair (contiguous 1 KB rows)
            xy_tiles = []
            for q in range(PAIRS_PER_CORE):
                xyq = consts.tile([128, 2, 2, P], dt.float8e4, tag=f"xy{q}")
                nc.sync.dma_start(xyq, xy_d[q])
                xy_tiles.append(xyq)

            utps_pool = ctx.enter_context(tc.tile_pool(name="utps", bufs=1, space="PSUM"))
            fps_pool = ctx.enter_context(tc.tile_pool(name="fps", bufs=1, space="PSUM"))
            dot_pool = ctx.enter_context(tc.tile_pool(name="dot", bufs=2, space="PSUM"))
            utsb_pool = ctx.enter_context(tc.tile_pool(name="utsb", bufs=8))
            elem_pool = ctx.enter_context(tc.tile_pool(name="elem", bufs=12))

            def step1(p, w_i):
                """DFT step 1 for pair p, input w_i (0=x,1=y): Ut[n, k0..255]."""
                if p == 0 and w_i == 1:
                    # lead-in: borrow the (still empty) fy PSUM buffer so
                    # step1-y doesn't WAR-wait on the utx drain; the fy ring
                    # naturally orders the first fy matmuls after this drain.
                    ut_ps = fps_pool.tile([128, 2, 2, 256], dt.float32, tag="fy")
                else:
                    ut_ps = utps_pool.tile([128, 2, 2, 256], dt.float32, tag="utps")
                for eo in range(2):
                    lhsT = xy_tiles[p][:, w_i, :, eo:P:2]
                    for comp in range(2):
                        nc.tensor.matmul(
                            ut_ps[:, comp, eo, :],
                            lhsT,
                            at_sb[:, comp],
                            start=True,
                            stop=True,
                            perf_mode=mybir.MatmulPerfMode.DoubleRow,
                        )
                # PSUM -> SBUF fp8; [p=n%128, comp, even/odd, k]
                ut_sb = utsb_pool.tile([128, 2, 2, 256], dt.float8e4, tag=f"utsb{w_i}")
                if p == 0 and w_i == 0:
                    # warmup: whole utx drain on ACT (idle then); with step1-y
                    # borrowing the fy buffer there is no WAR to hide anymore,
                    # and DVE stays free for the uty drain
                    nc.scalar.copy(ut_sb, ut_ps)
                else:
                    nc.vector.tensor_copy(ut_sb, ut_ps)
                return ut_sb

            def step2_lt(utx, uty, lt):
                """DFT step 2 for l-chunk lt: fields [l%128, w, pm, oc, k] in PSUM."""
                fx_ps = fps_pool.tile([128, 2, 2, 256], dt.float32, tag="fx")
                fy_ps = fps_pool.tile([128, 2, 2, 256], dt.float32, tag="fy")
                for pm in range(2):
                    for oc in range(2):
                        for comp in range(2):
                            lhsT = b_sb[:, lt, pm * 2 + oc, comp]
                            nc.tensor.matmul(
                                fx_ps[:, pm, oc, :], lhsT, utx[:, comp],
                                start=(comp == 0), stop=(comp == 1),
                                perf_mode=mybir.MatmulPerfMode.DoubleRow,
                            )
                            nc.tensor.matmul(
                                fy_ps[:, pm, oc, :], lhsT, uty[:, comp],
                                start=(comp == 0), stop=(comp == 1),
                                perf_mode=mybir.MatmulPerfMode.DoubleRow,
                            )
                return fx_ps, fy_ps

            def elem_a(p, lt, fx_ps, fy_ps):
                # stage A: drain PSUM -> SBUF bf16, square, |Fx|^2
                cx = elem_pool.tile([128, 2, 2, 256], dt.bfloat16, tag="cx")
                nc.scalar.copy(cx, fx_ps)
                cy = elem_pool.tile([128, 2, 2, 256], dt.bfloat16, tag="cy")
                nc.scalar.copy(cy, fy_ps)
                sq = elem_pool.tile([128, 2, 2, 256], dt.bfloat16, tag="sq")
                if p == PAIRS_PER_CORE - 1 and lt == 1:
                    # drain phase: keep the serial sq->a->rw chain on DVE to
                    # skip two cross-engine semaphore hops
                    nc.vector.tensor_mul(sq, cx, cx)
                else:
                    nc.gpsimd.tensor_mul(sq, cx, cx)
                a_sb = elem_pool.tile([128, 2, 256], dt.bfloat16, tag="a")
                nc.vector.tensor_add(a_sb, sq[:, :, 0], sq[:, :, 1])
                return cx, cy, a_sb

            def elem_b(state):
                # stage B: rw ~= 1/a via the bf16 bit-hack (magic - bits,
                # ~3% rel err -- the loss tolerates ~100x more), ry = Fy * rw
                p, lt, cx, cy, a_sb = state
                rw = elem_pool.tile([128, 2, 256], dt.bfloat16, tag="rw")
                nc.vector.tensor_scalar(
                    rw.bitcast(dt.uint16), a_sb.bitcast(dt.uint16),
                    -1.0, float(0x7EF0), op0=ALU.mult, op1=ALU.add,
                )
                ry = elem_pool.tile([128, 2, 2, 256], dt.bfloat16, tag="ry")
                if p == PAIRS_PER_CORE - 1 and lt == 1:
                    # drain phase: DVE is idle here and the two POOL TTs would
                    # serialize on the tail critical path
                    nc.vector.tensor_mul(ry[:, :, 0], cy[:, :, 0], rw)
                    nc.gpsimd.tensor_mul(ry[:, :, 1], cy[:, :, 1], rw)
                else:
                    nc.gpsimd.tensor_mul(ry[:, :, 0], cy[:, :, 0], rw)
                    nc.gpsimd.tensor_mul(ry[:, :, 1], cy[:, :, 1], rw)
                return p, lt, cx, cy, ry

            pair_d = {}

            def elem_c(state):
                # stage C: Gram-accumulate dots for this chunk; both l-chunks
                # of a pair share one PSUM bank, extracted once per pair.
                # Tail shortcut: the LAST pair stops its Gram after l-chunk 0
                # (extraction overlaps chunk 1) and computes chunk 1's dots as
                # two direct STT reductions -- no PSUM Gram / extraction on
                # the critical tail path.
                p, lt, cx, cy, ry = state
                c0 = p * 8
                _elem_c_body(p, lt, cx, cy, ry, c0)

            def _elem_c_body(p, lt, cx, cy, ry, c0):
                last = p == PAIRS_PER_CORE - 1
                if last and lt == 1:
                    # tail shortcut: no PSUM Gram / staging copy / big DMA on
                    # the critical tail -- chunk 1's dots are two direct STT
                    # reductions (SBUF operands, so Pool is legal) running
                    # Pool || DVE, each shipping a [128,1] column.
                    # NOTE: TensorScalarPtr is not a legal Pool opcode on HW
                    # (codegen engine check) -- both STTs must ride DVE.
                    tr1 = elem_pool.tile([128, 2, 2, 256], dt.bfloat16, tag="tr1")
                    col1 = elem_pool.tile([128, 1], dt.float32, tag="col1")
                    nc.vector.scalar_tensor_tensor(
                        tr1, cx, 1.0, ry, op0=ALU.mult, op1=ALU.mult,
                        accum_out=col1,
                    )
                    tr2 = elem_pool.tile([128, 2, 2, 256], dt.bfloat16, tag="tr2")
                    col2 = elem_pool.tile([128, 1], dt.float32, tag="col2")
                    nc.vector.scalar_tensor_tensor(
                        tr2, cy, 1.0, ry, op0=ALU.mult, op1=ALU.mult,
                        accum_out=col2,
                    )
                    nc.sync.dma_start(out2_d[0], col1)
                    nc.scalar.dma_start(out2_d[1], col2)
                    return
                if lt == 0:
                    d_tile = dot_pool.tile([128, 2, 128], dt.float32, tag="d")
                    pair_d[p] = d_tile
                d = pair_d[p]
                # S1 group first (oc=0 blocks lead: their ry half lands first).
                order = (0, 1, 4, 5, 2, 3, 6, 7)
                for which in range(2):
                    op = cx if which == 0 else cy
                    for i, c in enumerate(order):
                        a_i, b_i, h = c // 4, (c // 2) % 2, c % 2
                        opc = op[:, a_i, b_i, h * 128 : (h + 1) * 128]
                        ryc = ry[:, a_i, b_i, h * 128 : (h + 1) * 128]
                        st = (lt == 0 and i == 0)
                        sp = (lt == 1 or last) and i == 7
                        nc.tensor.matmul(d[:, which], opc, ryc, start=st, stop=sp)
                if lt == 1 or last:
                    # ship the raw Gram; the host extracts the two diagonals.
                    # The last pair's (chunk-0-only) Gram ships mid-stream,
                    # on ACT so it never blocks the tail chain's DVE ops.
                    dsb = elem_pool.tile([128, 2, 128], dt.float32, tag="dsb")
                    if last:
                        nc.scalar.copy(dsb, d)
                    else:
                        nc.vector.tensor_copy(dsb, d)
                    nc.sync.dma_start(out_d[p], dsb)
                    del pair_d[p]

            def chunk_a(p, utx, uty, lt):
                fx_ps, fy_ps = step2_lt(utx, uty, lt)
                return (p, lt) + elem_a(p, lt, fx_ps, fy_ps)

            # 3-deep software pipeline over the 12 (pair, lt) chunks:
            # A(c) | B(c-1) | C(c-2), with step1 of the next pair interleaved.
            # advance takes a THUNK so the older chunks' dots/extractions are
            # emitted (and queued) BEFORE the new chunk's step2/drains --
            # frees dot PSUM banks earlier and keeps the tail DVE chain clean.
            stage_b = stage_c = None

            def advance(state_a):
                nonlocal stage_b, stage_c
                if stage_c is not None:
                    elem_c(stage_c)
                    stage_c = None
                if stage_b is not None:
                    stage_c = elem_b(stage_b)
                stage_b = state_a

            pending = None
            for p in range(PAIRS_PER_CORE):
                # chunk 0 of the pending pair is emitted BEFORE this pair's
                # step1 so its step2 matmuls win the PE priority tie against
                # the (slack-rich) step1 -- pulls every pair's first drain in
                if pending is not None:
                    advance(chunk_a(pending[0], pending[1], pending[2], 0))
                utx = step1(p, 0)
                if pending is not None:
                    advance(chunk_a(pending[0], pending[1], pending[2], 1))
                uty = step1(p, 1)
                pending = (p, utx, uty)
            q, ux, uy = pending
            advance(chunk_a(q, ux, uy, 0))
            advance(chunk_a(q, ux, uy, 1))
            advance(None)
            if stage_c is not None:
                elem_c(stage_c)

    nc.finalize()  # Bacc: runs wait-splitting (1-wait/inst HW limit), reg alloc
    _NC = nc
    return nc


# ---------------------------------------------------------------- host side
def _host_corrections(x, y, A_full):
    """Exact (float64) k=0-row and l=0-col sums of cr/a and b/a for one pair."""
    x = x.astype(np.float64)
    y = y.astype(np.float64)
    # l=0 column: F[k,0] = A @ row-sums (sum over n)
    Fx0 = A_full @ x.sum(axis=1)
    Fy0 = A_full @ y.sum(axis=1)
    a0 = np.abs(Fx0) ** 2
    s1c = ((np.conj(Fx0) * Fy0).real / a0).sum()
    s2c = (np.abs(Fy0) ** 2 / a0).sum()
    # k=0 row, l=0..255: F[0,l] = A[:256] @ col-sums (sum over m)
    Fx1 = A_full[:256] @ x.sum(axis=0)
    Fy1 = A_full[:256] @ y.sum(axis=0)
    a1 = np.abs(Fx1) ** 2
    s1r = ((np.conj(Fx1) * Fy1).real / a1).sum()
    s2r = (np.abs(Fy1) ** 2 / a1).sum()
    return s1c, s2c, s1r, s2r


def kernel(recon, target):
    atc, bstk, ident, A_full = _consts()
    f8 = mybir.dt.np(dt.float8e4)
    xs = target.reshape(48, 128, 2, P).astype(f8)  # x = target; m -> (i, j)
    ys = recon.reshape(48, 128, 2, P).astype(f8)  # y = recon
    xy = np.ascontiguousarray(np.stack([xs, ys], axis=2))  # [48, 128, 2, 2, 256]

    nc = _build()
    in_maps = [
        {
            "xy": xy[c * PAIRS_PER_CORE : (c + 1) * PAIRS_PER_CORE],
            "atc": atc,
            "bstk": bstk,
        }
        for c in range(NCORES)
    ]
    res = None
    for attempt in range(3):
        try:
            res = run_bass_kernel_spmd(nc, in_maps, core_ids=list(range(NCORES)))
            break
        except Exception:
            if attempt == 2:
                raise
            import time as _time

            _time.sleep(2.0)

    NN = float(N) * float(N)
    loss = 0.0
    for c in range(NCORES):
        grams = res.results[c]["out"].astype(np.float64)  # [6, 128, 2, 128]
        cols2 = res.results[c]["out2"].astype(np.float64)  # [2, 128, 1]
        kk = np.arange(128)
        for p in range(PAIRS_PER_CORE):
            s1_dev = grams[p, kk, 0, kk]
            s2_dev = grams[p, kk, 1, kk]
            if p == PAIRS_PER_CORE - 1:
                # tail shortcut: last pair's Gram covers chunk 0 only;
                # chunk 1's dots arrive as STT accumulation columns
                s1_dev = np.concatenate([s1_dev, cols2[0, :, 0]])
                s2_dev = np.concatenate([s2_dev, cols2[1, :, 0]])
            pair = c * PAIRS_PER_CORE + p
            b, ch = divmod(pair, 3)
            s1c, s2c, s1r, s2r = _host_corrections(
                np.asarray(target[b, ch]), np.asarray(recon[b, ch]), A_full
            )
            S1 = 2.0 * (s1_dev.sum() - s1r) - s1c
            S2 = 2.0 * (s2_dev.sum() - s2r) - s2c
            v00 = S1 / NN
            E = S2 / NN
            loss += 0.5 * (1.0 - v00 * v00 / E)
    return np.float32(loss)



# revision 64
# speedup vs baseline: 1.0704x; 1.0348x over previous
"""AWLoss (adaptive Wiener filter loss) Trainium2 kernel, 8-core data-parallel.

Math (analytic reduction verified against the reference to ~2e-8 rel err):
  The penalty T (std=1e-4) is numerically 1 everywhere except 0 at the center
  pixel, and the roll puts that center at pre-roll [0,0]. The loss collapses to
      loss = sum_{b,c} 0.5 * (1 - v00^2 / E)
  with, per (b,c) pair (x = target, y = recon, A = 511x256 padded-DFT matrix):
      Fx = A x A^T,  Fy = A y A^T          (full 511x511 spectra)
      a = |Fx|^2, cr = Re(conj(Fx) Fy), b = |Fy|^2
      v00 = sum(cr/a) / N^2,   E = sum(b/a) / N^2       (N = 511)
  The flip-phase factor cancels between numerator and denominator and the
  eps=1e-9 pre-whitening is negligible (|Fx|^2 >~ 0.5 everywhere).

Spectral coverage: the device computes q-sums over rows k = 0..255 ("+"
fields) and rows (511-k) mod 511 ("-" fields, via conjugated DFT stacks
re-using the same Ut), columns l = 0..255. With Hermitian symmetry
  S_full = 2*(S_device - S_row0_half) - S_col0
where the k=0 row (duplicated on device) and l=0 column sums are recomputed
exactly on the host from row/column sums of x and y (tiny 1-D DFTs).

Device pipeline, per (b,c) pair, software-pipelined over 12 (pair, l-chunk)
chunks (all matmuls fp8-e4m3 with DoubleRow pairing, fp32 PSUM accum):
  step1 (PE):  Ut[n,k] = sum_m x[m,n] A[k,m], k = 0..255
  ut copy (DVE): PSUM -> SBUF fp8
  step2 (PE):  F(+-)[l,k] = sum_n B(+-)[n,l] Ut[n,k], per l-chunk of 128
  cx/cy (ACT): F fields PSUM -> SBUF bf16
  sq,ry (POOL), a (DVE): |Fx|^2 and ry = Fy * rw
  rw (DVE):    1/a via the bf16 bit-hack (0x7EF0 - bits, ~3% err; the loss
               term tolerates ~100x more)
  dots (PE):   S1 = sum cx.ry, S2 = sum cy.ry as Gram-matrix accumulations
               over 128x128 chunks; diagonal extracted with a
               scalar_tensor_tensor against the identity (accum_out).
Host: Hermitian corrections, v00/E ratios, final sum in float64.

Toolchain notes: bacc.Bacc + explicit finalize() (walrus allows at most one
sem wait per instruction; Bacc's generate_event_semaphores splits them).
Engine-legality notes (BIR verifier): GPSIMD/Pool cannot touch PSUM (all
PSUM drains must be ACT or DVE), and dma_start cannot source PSUM -- both
constraints shape the drain/dot structure above. Lead-in: pair-0's step1-y
borrows the empty fy PSUM buffer so it needn't WAR-wait on the utx drain
(the fy ring then orders the first fy matmuls after the uty drain).
"""

import os
import sys
from contextlib import ExitStack
from functools import lru_cache

import numpy as np

sys.path.insert(0, "/opt/trn_rl_repo")

import ml_dtypes

import concourse.bacc as bacc
import concourse.mybir as mybir
import concourse.tile as tile
from concourse.bass_utils import run_bass_kernel_spmd

dt = mybir.dt
ALU = mybir.AluOpType

N = 511
OFF = 127
P = 256
NCORES = 8
PAIRS_PER_CORE = 6  # 48 (b,c) pairs / 8 cores


# ---------------------------------------------------------------- constants
@lru_cache(maxsize=1)
def _consts():
    k = np.arange(256)
    m = np.arange(P)
    # step1 rhs AT[m, k] = A[k, m] = exp(-2pi i k (m+OFF) / N), k = 0..255
    theta = 2.0 * np.pi * np.outer(k, (m + OFF)) / N  # [k, m]
    atc = np.stack([np.cos(theta).T, -np.sin(theta).T])  # [2, m, k]
    atc = atc.reshape(2, 128, 2, 256)  # m -> (i, j) DoubleRow pairs
    # step2 lhsT B[n, l] = A[l, n]: Br = cos(2pi l (n+OFF)/N), Bi = -sin
    phi = 2.0 * np.pi * np.outer(m + OFF, np.arange(256)) / N  # [n, l]
    Br = np.cos(phi)
    Bi = -np.sin(phi)
    # stacked lhsT over contraction rows s: s=0,1 multiply Utr, s=2,3 Uti.
    # F+ = (Br + iBi).(Utr + iUti):  F+r = Br.Utr - Bi.Uti ; F+i = Bi.Utr + Br.Uti
    # F- = (Br + iBi).conj(Ut):      F-r = Br.Utr + Bi.Uti ; F-i = Bi.Utr - Br.Uti
    b2 = np.empty((2, 2, 2, 256, 256))  # [pm, oc, comp(of Ut), n, l]
    for pm, sgn in ((0, 1.0), (1, -1.0)):
        b2[pm, 0, 0] = Br
        b2[pm, 1, 0] = Bi
        b2[pm, 0, 1] = -sgn * Bi
        b2[pm, 1, 1] = sgn * Br
    # DoubleRow pairing n = 2i + j: [pm, oc, comp, i, j, l] -> [i, lt, (pm oc), comp, j, l%128]
    # flattened per partition so the load DMA is one contiguous 4 KB run
    # (>=512 B inner run avoids the cost model's 2x small-elem latency mult).
    b2 = b2.reshape(2, 2, 2, 128, 2, 2, 128)  # [pm, oc, comp, i, j, lt, l128]
    bstk2 = np.ascontiguousarray(
        np.transpose(b2, (3, 5, 0, 1, 2, 4, 6)).reshape(128, 4096)
    )
    bf16 = ml_dtypes.bfloat16
    f8 = mybir.dt.np(dt.float8e4)
    # full complex A (float64) for host-side row/col corrections
    kk = np.arange(N)
    A_full = np.exp(-2j * np.pi * np.outer(kk, (m + OFF)) / N)  # [511, 256]
    ident = np.eye(128)
    return atc.astype(f8), bstk2.astype(f8), ident.astype(bf16), A_full


# ---------------------------------------------------------------- bass build
_NC = None


def _build():
    global _NC
    if _NC is not None:
        return _NC
    nc = bacc.Bacc()
    xy_d = nc.dram_tensor("xy", [PAIRS_PER_CORE, 128, 2, 2, P], dt.float8e4, kind="ExternalInput")
    atc_d = nc.dram_tensor("atc", [2, 128, 2, 256], dt.float8e4, kind="ExternalInput")
    bstk_d = nc.dram_tensor("bstk", [128, 4096], dt.float8e4, kind="ExternalInput")
    out_d = nc.dram_tensor("out", [PAIRS_PER_CORE, 128, 2, 128], dt.float32, kind="ExternalOutput")
    # last pair's chunk-1 dots (tail shortcut): slot 0 = raw S1 mini-Gram,
    # slot 1 (column 0) = S2 STT accumulation column
    out2_d = nc.dram_tensor("out2", [2, 128, 128], dt.float32, kind="ExternalOutput")

    with tile.TileContext(nc) as tc:
        with ExitStack() as ctx:
            consts = ctx.enter_context(tc.tile_pool(name="consts", bufs=1))
            # ACT table prime: pay the 1283 ns activation-table load at t~0
            # on the idle ACT engine instead of inside the first real drain.
            prime = consts.tile([128, 1], dt.bfloat16, tag="prime")
            prime2 = consts.tile([128, 1], dt.bfloat16, tag="prime2")
            nc.gpsimd.memset(prime, 0.0)
            nc.scalar.copy(prime2, prime)
            # AT tiles (fp8 DoubleRow): [p=i, comp, j, k] on the ACT hwdge
            # queue so it doesn't serialize behind the pair-0 input DMA.
            at_sb = consts.tile([128, 2, 2, 256], dt.float8e4)
            nc.scalar.dma_start(at_sb, atc_d.rearrange("c p j k -> p c j k"))
            # B tiles (fp8 DoubleRow): [p=i, lt, g=(pm,oc), comp, j, l%128],
            # shipped pre-flattened so the load is ONE contiguous 4 KB/row DMA
            # (1x latency class) on the gpsimd queue.
            b_sb = consts.tile([128, 2, 4, 2, 2, 128], dt.float8e4)
            nc.gpsimd.dma_start(b_sb, bstk_d[:])
            # inputs (fp8 DoubleRow): per-pair tiles [p=i, which, j, n], m = 2i+j
            # one combined x+y DMA per pair (contiguous 1 KB rows)
            xy_tiles = []
            for q in range(PAIRS_PER_CORE):
                xyq = consts.tile([128, 2, 2, P], dt.float8e4, tag=f"xy{q}")
                nc.sync.dma_start(xyq, xy_d[q])
                xy_tiles.append(xyq)

            utps_pool = ctx.enter_context(tc.tile_pool(name="utps", bufs=1, space="PSUM"))
            fps_pool = ctx.enter_context(tc.tile_pool(name="fps", bufs=1, space="PSUM"))
            dot_pool = ctx.enter_context(tc.tile_pool(name="dot", bufs=2, space="PSUM"))
            utsb_pool = ctx.enter_context(tc.tile_pool(name="utsb", bufs=8))
            elem_pool = ctx.enter_context(tc.tile_pool(name="elem", bufs=12))

            def step1(p, w_i):
                """DFT step 1 for pair p, input w_i (0=x,1=y): Ut[n, k0..255]."""
                if p == 0 and w_i == 1:
                    # lead-in: borrow the (still empty) fy PSUM buffer so
                    # step1-y doesn't WAR-wait on the utx drain; the fy ring
                    # naturally orders the first fy matmuls after this drain.
                    ut_ps = fps_pool.tile([128, 2, 2, 256], dt.float32, tag="fy")
                else:
                    ut_ps = utps_pool.tile([128, 2, 2, 256], dt.float32, tag="utps")
                for eo in range(2):
                    lhsT = xy_tiles[p][:, w_i, :, eo:P:2]
                    for comp in range(2):
                        nc.tensor.matmul(
                            ut_ps[:, comp, eo, :],
                            lhsT,
                            at_sb[:, comp],
                            start=True,
                            stop=True,
                            perf_mode=mybir.MatmulPerfMode.DoubleRow,
                        )
                # PSUM -> SBUF fp8; [p=n%128, comp, even/odd, k]
                ut_sb = utsb_pool.tile([128, 2, 2, 256], dt.float8e4, tag=f"utsb{w_i}")
                if p == 0 and w_i == 0:
                    # warmup: whole utx drain on ACT (idle then); with step1-y
                    # borrowing the fy buffer there is no WAR to hide anymore,
                    # and DVE stays free for the uty drain
                    nc.scalar.copy(ut_sb, ut_ps)
                else:
                    nc.vector.tensor_copy(ut_sb, ut_ps)
                return ut_sb

            def step2_lt(utx, uty, lt):
                """DFT step 2 for l-chunk lt: fields [l%128, w, pm, oc, k] in PSUM."""
                fx_ps = fps_pool.tile([128, 2, 2, 256], dt.float32, tag="fx")
                fy_ps = fps_pool.tile([128, 2, 2, 256], dt.float32, tag="fy")
                for pm in range(2):
                    for oc in range(2):
                        for comp in range(2):
                            lhsT = b_sb[:, lt, pm * 2 + oc, comp]
                            nc.tensor.matmul(
                                fx_ps[:, pm, oc, :], lhsT, utx[:, comp],
                                start=(comp == 0), stop=(comp == 1),
                                perf_mode=mybir.MatmulPerfMode.DoubleRow,
                            )
                            nc.tensor.matmul(
                                fy_ps[:, pm, oc, :], lhsT, uty[:, comp],
                                start=(comp == 0), stop=(comp == 1),
                                perf_mode=mybir.MatmulPerfMode.DoubleRow,
                            )
                return fx_ps, fy_ps

            def elem_a(p, lt, fx_ps, fy_ps):
                # stage A: drain PSUM -> SBUF bf16, square, |Fx|^2
                cx = elem_pool.tile([128, 2, 2, 256], dt.bfloat16, tag="cx")
                nc.scalar.copy(cx, fx_ps)
                cy = elem_pool.tile([128, 2, 2, 256], dt.bfloat16, tag="cy")
                nc.scalar.copy(cy, fy_ps)
                sq = elem_pool.tile([128, 2, 2, 256], dt.bfloat16, tag="sq")
                if p == PAIRS_PER_CORE - 1 and lt == 1:
                    # drain phase: keep the serial sq->a->rw chain on DVE to
                    # skip two cross-engine semaphore hops
                    nc.vector.tensor_mul(sq, cx, cx)
                else:
                    nc.gpsimd.tensor_mul(sq, cx, cx)
                a_sb = elem_pool.tile([128, 2, 256], dt.bfloat16, tag="a")
                nc.vector.tensor_add(a_sb, sq[:, :, 0], sq[:, :, 1])
                return cx, cy, a_sb

            def elem_b(state):
                # stage B: rw ~= 1/a via the bf16 bit-hack (magic - bits,
                # ~3% rel err -- the loss tolerates ~100x more), ry = Fy * rw
                p, lt, cx, cy, a_sb = state
                rw = elem_pool.tile([128, 2, 256], dt.bfloat16, tag="rw")
                nc.vector.tensor_scalar(
                    rw.bitcast(dt.uint16), a_sb.bitcast(dt.uint16),
                    -1.0, float(0x7EF0), op0=ALU.mult, op1=ALU.add,
                )
                ry = elem_pool.tile([128, 2, 2, 256], dt.bfloat16, tag="ry")
                if p == PAIRS_PER_CORE - 1 and lt == 1:
                    # drain phase: DVE is idle here and the two POOL TTs would
                    # serialize on the tail critical path
                    nc.vector.tensor_mul(ry[:, :, 0], cy[:, :, 0], rw)
                    nc.gpsimd.tensor_mul(ry[:, :, 1], cy[:, :, 1], rw)
                else:
                    nc.gpsimd.tensor_mul(ry[:, :, 0], cy[:, :, 0], rw)
                    nc.gpsimd.tensor_mul(ry[:, :, 1], cy[:, :, 1], rw)
                return p, lt, cx, cy, ry

            pair_d = {}

            def elem_c(state):
                # stage C: Gram-accumulate dots for this chunk; both l-chunks
                # of a pair share one PSUM bank, extracted once per pair.
                # Tail shortcut: the LAST pair stops its Gram after l-chunk 0
                # (extraction overlaps chunk 1) and computes chunk 1's dots as
                # two direct STT reductions -- no PSUM Gram / extraction on
                # the critical tail path.
                p, lt, cx, cy, ry = state
                c0 = p * 8
                _elem_c_body(p, lt, cx, cy, ry, c0)

            def _elem_c_body(p, lt, cx, cy, ry, c0):
                last = p == PAIRS_PER_CORE - 1
                if last and lt == 1:
                    # tail shortcut, hybrid: S1 as a PE mini-Gram (8 matmuls,
                    # can start on the first ry half) in PARALLEL with S2 as
                    # one DVE STT reduction (TensorScalarPtr is not a legal
                    # Pool opcode on HW, so only one STT rides DVE).
                    # reuse the d-ring (same shape) for the S1 mini-Gram
                    d2 = dot_pool.tile([128, 2, 128], dt.float32, tag="d")
                    order = (0, 1, 4, 5, 2, 3, 6, 7)
                    for i, c in enumerate(order):
                        a_i, b_i, h = c // 4, (c // 2) % 2, c % 2
                        nc.tensor.matmul(
                            d2[:, 0, :],
                            cx[:, a_i, b_i, h * 128 : (h + 1) * 128],
                            ry[:, a_i, b_i, h * 128 : (h + 1) * 128],
                            start=(i == 0), stop=(i == 7),
                        )
                    tr2 = elem_pool.tile([128, 2, 2, 256], dt.bfloat16, tag="tr2")
                    col2 = elem_pool.tile([128, 1], dt.float32, tag="col2")
                    nc.vector.scalar_tensor_tensor(
                        tr2, cy, 1.0, ry, op0=ALU.mult, op1=ALU.mult,
                        accum_out=col2,
                    )
                    dsb2 = elem_pool.tile([128, 128], dt.float32, tag="dsb2")
                    nc.scalar.copy(dsb2, d2[:, 0, :])  # ACT is idle on the tail
                    nc.sync.dma_start(out2_d[0], dsb2)
                    nc.scalar.dma_start(out2_d[1, :, 0:1], col2)
                    return
                if lt == 0:
                    d_tile = dot_pool.tile([128, 2, 128], dt.float32, tag="d")
                    pair_d[p] = d_tile
                d = pair_d[p]
                # S1 group first (oc=0 blocks lead: their ry half lands first).
                order = (0, 1, 4, 5, 2, 3, 6, 7)
                for which in range(2):
                    op = cx if which == 0 else cy
                    for i, c in enumerate(order):
                        a_i, b_i, h = c // 4, (c // 2) % 2, c % 2
                        opc = op[:, a_i, b_i, h * 128 : (h + 1) * 128]
                        ryc = ry[:, a_i, b_i, h * 128 : (h + 1) * 128]
                        st = (lt == 0 and i == 0)
                        sp = (lt == 1 or last) and i == 7
                        nc.tensor.matmul(d[:, which], opc, ryc, start=st, stop=sp)
                if lt == 1 or last:
                    # ship the raw Gram; the host extracts the two diagonals.
                    # The last pair's (chunk-0-only) Gram ships mid-stream,
                    # on ACT so it never blocks the tail chain's DVE ops.
                    dsb = elem_pool.tile([128, 2, 128], dt.float32, tag="dsb")
                    if last:
                        nc.scalar.copy(dsb, d)
                    else:
                        nc.vector.tensor_copy(dsb, d)
                    nc.sync.dma_start(out_d[p], dsb)
                    del pair_d[p]

            def chunk_a(p, utx, uty, lt):
                fx_ps, fy_ps = step2_lt(utx, uty, lt)
                return (p, lt) + elem_a(p, lt, fx_ps, fy_ps)

            # 3-deep software pipeline over the 12 (pair, lt) chunks:
            # A(c) | B(c-1) | C(c-2), with step1 of the next pair interleaved.
            # advance takes a THUNK so the older chunks' dots/extractions are
            # emitted (and queued) BEFORE the new chunk's step2/drains --
            # frees dot PSUM banks earlier and keeps the tail DVE chain clean.
            stage_b = stage_c = None

            def advance(state_a):
                nonlocal stage_b, stage_c
                if stage_c is not None:
                    elem_c(stage_c)
                    stage_c = None
                if stage_b is not None:
                    stage_c = elem_b(stage_b)
                stage_b = state_a

            pending = None
            for p in range(PAIRS_PER_CORE):
                # chunk 0 of the pending pair is emitted BEFORE this pair's
                # step1 so its step2 matmuls win the PE priority tie against
                # the (slack-rich) step1 -- pulls every pair's first drain in
                if pending is not None:
                    advance(chunk_a(pending[0], pending[1], pending[2], 0))
                utx = step1(p, 0)
                if pending is not None:
                    advance(chunk_a(pending[0], pending[1], pending[2], 1))
                uty = step1(p, 1)
                pending = (p, utx, uty)
            q, ux, uy = pending
            advance(chunk_a(q, ux, uy, 0))
            advance(chunk_a(q, ux, uy, 1))
            advance(None)
            if stage_c is not None:
                elem_c(stage_c)

    nc.finalize()  # Bacc: runs wait-splitting (1-wait/inst HW limit), reg alloc
    _NC = nc
    return nc


# ---------------------------------------------------------------- host side
def _host_corrections(x, y, A_full):
    """Exact (float64) k=0-row and l=0-col sums of cr/a and b/a for one pair."""
    x = x.astype(np.float64)
    y = y.astype(np.float64)
    # l=0 column: F[k,0] = A @ row-sums (sum over n)
    Fx0 = A_full @ x.sum(axis=1)
    Fy0 = A_full @ y.sum(axis=1)
    a0 = np.abs(Fx0) ** 2
    s1c = ((np.conj(Fx0) * Fy0).real / a0).sum()
    s2c = (np.abs(Fy0) ** 2 / a0).sum()
    # k=0 row, l=0..255: F[0,l] = A[:256] @ col-sums (sum over m)
    Fx1 = A_full[:256] @ x.sum(axis=0)
    Fy1 = A_full[:256] @ y.sum(axis=0)
    a1 = np.abs(Fx1) ** 2
    s1r = ((np.conj(Fx1) * Fy1).real / a1).sum()
    s2r = (np.abs(Fy1) ** 2 / a1).sum()
    return s1c, s2c, s1r, s2r


def kernel(recon, target):
    atc, bstk, ident, A_full = _consts()
    f8 = mybir.dt.np(dt.float8e4)
    xs = target.reshape(48, 128, 2, P).astype(f8)  # x = target; m -> (i, j)
    ys = recon.reshape(48, 128, 2, P).astype(f8)  # y = recon
    xy = np.ascontiguousarray(np.stack([xs, ys], axis=2))  # [48, 128, 2, 2, 256]

    nc = _build()
    in_maps = [
        {
            "xy": xy[c * PAIRS_PER_CORE : (c + 1) * PAIRS_PER_CORE],
            "atc": atc,
            "bstk": bstk,
        }
        for c in range(NCORES)
    ]
    res = None
    for attempt in range(3):
        try:
            res = run_bass_kernel_spmd(nc, in_maps, core_ids=list(range(NCORES)))
            break
        except Exception:
            if attempt == 2:
                raise
            import time as _time

            _time.sleep(2.0)

    NN = float(N) * float(N)
    loss = 0.0
    for c in range(NCORES):
        grams = res.results[c]["out"].astype(np.float64)  # [6, 128, 2, 128]
        cols2 = res.results[c]["out2"].astype(np.float64)  # [2, 128, 1]
        kk = np.arange(128)
        for p in range(PAIRS_PER_CORE):
            s1_dev = grams[p, kk, 0, kk]
            s2_dev = grams[p, kk, 1, kk]
            if p == PAIRS_PER_CORE - 1:
                # tail shortcut: last pair's Gram covers chunk 0 only;
                # chunk 1 arrives as a raw S1 mini-Gram + an S2 column
                s1_dev = np.concatenate([s1_dev, cols2[0, kk, kk]])
                s2_dev = np.concatenate([s2_dev, cols2[1, :, 0]])
            pair = c * PAIRS_PER_CORE + p
            b, ch = divmod(pair, 3)
            s1c, s2c, s1r, s2r = _host_corrections(
                np.asarray(target[b, ch]), np.asarray(recon[b, ch]), A_full
            )
            S1 = 2.0 * (s1_dev.sum() - s1r) - s1c
            S2 = 2.0 * (s2_dev.sum() - s2r) - s2c
            v00 = S1 / NN
            E = S2 / NN
            loss += 0.5 * (1.0 - v00 * v00 / E)
    return np.float32(loss)



# revision 69
# speedup vs baseline: 1.0818x; 1.0106x over previous
"""AWLoss (adaptive Wiener filter loss) Trainium2 kernel, 8-core data-parallel.

Math (analytic reduction verified against the reference to ~2e-8 rel err):
  The penalty T (std=1e-4) is numerically 1 everywhere except 0 at the center
  pixel, and the roll puts that center at pre-roll [0,0]. The loss collapses to
      loss = sum_{b,c} 0.5 * (1 - v00^2 / E)
  with, per (b,c) pair (x = target, y = recon, A = 511x256 padded-DFT matrix):
      Fx = A x A^T,  Fy = A y A^T          (full 511x511 spectra)
      a = |Fx|^2, cr = Re(conj(Fx) Fy), b = |Fy|^2
      v00 = sum(cr/a) / N^2,   E = sum(b/a) / N^2       (N = 511)
  The flip-phase factor cancels between numerator and denominator and the
  eps=1e-9 pre-whitening is negligible (|Fx|^2 >~ 0.5 everywhere).

Spectral coverage: the device computes q-sums over rows k = 0..255 ("+"
fields) and rows (511-k) mod 511 ("-" fields, via conjugated DFT stacks
re-using the same Ut), columns l = 0..255. With Hermitian symmetry
  S_full = 2*(S_device - S_row0_half) - S_col0
where the k=0 row (duplicated on device) and l=0 column sums are recomputed
exactly on the host from row/column sums of x and y (tiny 1-D DFTs).

Device pipeline, per (b,c) pair, software-pipelined over 12 (pair, l-chunk)
chunks (all matmuls fp8-e4m3 with DoubleRow pairing, fp32 PSUM accum):
  step1 (PE):  Ut[n,k] = sum_m x[m,n] A[k,m], k = 0..255
  ut copy (DVE): PSUM -> SBUF fp8
  step2 (PE):  F(+-)[l,k] = sum_n B(+-)[n,l] Ut[n,k], per l-chunk of 128
  cx/cy (ACT): F fields PSUM -> SBUF bf16
  sq,ry (POOL), a (DVE): |Fx|^2 and ry = Fy * rw
  rw (DVE):    1/a via the bf16 bit-hack (0x7EF0 - bits, ~3% err; the loss
               term tolerates ~100x more)
  dots (PE):   S1 = sum cx.ry, S2 = sum cy.ry as Gram-matrix accumulations
               over 128x128 chunks; diagonal extracted with a
               scalar_tensor_tensor against the identity (accum_out).
Host: Hermitian corrections, v00/E ratios, final sum in float64.

Toolchain notes: bacc.Bacc + explicit finalize() (walrus allows at most one
sem wait per instruction; Bacc's generate_event_semaphores splits them).
Engine-legality notes (BIR verifier): GPSIMD/Pool cannot touch PSUM (all
PSUM drains must be ACT or DVE), and dma_start cannot source PSUM -- both
constraints shape the drain/dot structure above. Lead-in: pair-0's step1-y
borrows the empty fy PSUM buffer so it needn't WAR-wait on the utx drain
(the fy ring then orders the first fy matmuls after the uty drain).
"""

import os
import sys
from contextlib import ExitStack
from functools import lru_cache

import numpy as np

sys.path.insert(0, "/opt/trn_rl_repo")

import ml_dtypes

import concourse.bacc as bacc
import concourse.mybir as mybir
import concourse.tile as tile
from concourse.bass_utils import run_bass_kernel_spmd

dt = mybir.dt
ALU = mybir.AluOpType

N = 511
OFF = 127
P = 256
NCORES = 8
PAIRS_PER_CORE = 6  # 48 (b,c) pairs / 8 cores


# ---------------------------------------------------------------- constants
@lru_cache(maxsize=1)
def _consts():
    k = np.arange(256)
    m = np.arange(P)
    # step1 rhs AT[m, k] = A[k, m] = exp(-2pi i k (m+OFF) / N), k = 0..255
    theta = 2.0 * np.pi * np.outer(k, (m + OFF)) / N  # [k, m]
    atc = np.stack([np.cos(theta).T, -np.sin(theta).T])  # [2, m, k]
    atc = atc.reshape(2, 128, 2, 256)  # m -> (i, j) DoubleRow pairs
    # step2 lhsT B[n, l] = A[l, n]: Br = cos(2pi l (n+OFF)/N), Bi = -sin
    phi = 2.0 * np.pi * np.outer(m + OFF, np.arange(256)) / N  # [n, l]
    Br = np.cos(phi)
    Bi = -np.sin(phi)
    # stacked lhsT over contraction rows s: s=0,1 multiply Utr, s=2,3 Uti.
    # F+ = (Br + iBi).(Utr + iUti):  F+r = Br.Utr - Bi.Uti ; F+i = Bi.Utr + Br.Uti
    # F- = (Br + iBi).conj(Ut):      F-r = Br.Utr + Bi.Uti ; F-i = Bi.Utr - Br.Uti
    b2 = np.empty((2, 2, 2, 256, 256))  # [pm, oc, comp(of Ut), n, l]
    for pm, sgn in ((0, 1.0), (1, -1.0)):
        b2[pm, 0, 0] = Br
        b2[pm, 1, 0] = Bi
        b2[pm, 0, 1] = -sgn * Bi
        b2[pm, 1, 1] = sgn * Br
    # DoubleRow pairing n = 2i + j: [pm, oc, comp, i, j, l] -> [i, lt, (pm oc), comp, j, l%128]
    # flattened per partition so the load DMA is one contiguous 4 KB run
    # (>=512 B inner run avoids the cost model's 2x small-elem latency mult).
    b2 = b2.reshape(2, 2, 2, 128, 2, 2, 128)  # [pm, oc, comp, i, j, lt, l128]
    bstk2 = np.ascontiguousarray(
        np.transpose(b2, (3, 5, 0, 1, 2, 4, 6)).reshape(128, 4096)
    )
    bf16 = ml_dtypes.bfloat16
    f8 = mybir.dt.np(dt.float8e4)
    # full complex A (float64) for host-side row/col corrections
    kk = np.arange(N)
    A_full = np.exp(-2j * np.pi * np.outer(kk, (m + OFF)) / N)  # [511, 256]
    ident = np.eye(128)
    return atc.astype(f8), bstk2.astype(f8), ident.astype(bf16), A_full


# ---------------------------------------------------------------- bass build
_NC = None


def _build():
    global _NC
    if _NC is not None:
        return _NC
    nc = bacc.Bacc()
    xy_d = nc.dram_tensor("xy", [PAIRS_PER_CORE, 128, 2, 2, P], dt.float8e4, kind="ExternalInput")
    atc_d = nc.dram_tensor("atc", [2, 128, 2, 256], dt.float8e4, kind="ExternalInput")
    bstk_d = nc.dram_tensor("bstk", [128, 4096], dt.float8e4, kind="ExternalInput")
    out_d = nc.dram_tensor("out", [PAIRS_PER_CORE, 128, 2, 128], dt.float32, kind="ExternalOutput")
    # last pair's chunk-1 dots (tail shortcut): slot 0 = raw S1 mini-Gram,
    # slot 1 (column 0) = S2 STT accumulation column
    out2_d = nc.dram_tensor("out2", [2, 128, 128], dt.float32, kind="ExternalOutput")

    with tile.TileContext(nc) as tc:
        with ExitStack() as ctx:
            consts = ctx.enter_context(tc.tile_pool(name="consts", bufs=1))
            # ACT table prime: pay the 1283 ns activation-table load at t~0
            # on the idle ACT engine instead of inside the first real drain.
            prime = consts.tile([128, 1], dt.bfloat16, tag="prime")
            prime2 = consts.tile([128, 1], dt.bfloat16, tag="prime2")
            nc.gpsimd.memset(prime, 0.0)
            nc.scalar.copy(prime2, prime)
            # AT tiles (fp8 DoubleRow): [p=i, comp, j, k] on the ACT hwdge
            # queue so it doesn't serialize behind the pair-0 input DMA.
            at_sb = consts.tile([128, 2, 2, 256], dt.float8e4)
            nc.scalar.dma_start(at_sb, atc_d.rearrange("c p j k -> p c j k"))
            # B tiles (fp8 DoubleRow): [p=i, lt, g=(pm,oc), comp, j, l%128],
            # shipped pre-flattened so the load is ONE contiguous 4 KB/row DMA
            # (1x latency class) on the gpsimd queue.
            b_sb = consts.tile([128, 2, 4, 2, 2, 128], dt.float8e4)
            nc.gpsimd.dma_start(b_sb, bstk_d[:])
            # inputs (fp8 DoubleRow): per-pair tiles [p=i, which, j, n], m = 2i+j
            # one combined x+y DMA per pair (contiguous 1 KB rows)
            xy_tiles = []
            for q in range(PAIRS_PER_CORE):
                xyq = consts.tile([128, 2, 2, P], dt.float8e4, tag=f"xy{q}")
                nc.sync.dma_start(xyq, xy_d[q])
                xy_tiles.append(xyq)

            utps_pool = ctx.enter_context(tc.tile_pool(name="utps", bufs=1, space="PSUM"))
            fps_pool = ctx.enter_context(tc.tile_pool(name="fps", bufs=1, space="PSUM"))
            dot_pool = ctx.enter_context(tc.tile_pool(name="dot", bufs=2, space="PSUM"))
            utsb_pool = ctx.enter_context(tc.tile_pool(name="utsb", bufs=8))
            elem_pool = ctx.enter_context(tc.tile_pool(name="elem", bufs=12))

            def step1(p, w_i):
                """DFT step 1 for pair p, input w_i (0=x,1=y): Ut[n, k0..255]."""
                if p == 0 and w_i == 1:
                    # lead-in: borrow the (still empty) fy PSUM buffer so
                    # step1-y doesn't WAR-wait on the utx drain; the fy ring
                    # naturally orders the first fy matmuls after this drain.
                    ut_ps = fps_pool.tile([128, 2, 2, 256], dt.float32, tag="fy")
                else:
                    ut_ps = utps_pool.tile([128, 2, 2, 256], dt.float32, tag="utps")
                for eo in range(2):
                    lhsT = xy_tiles[p][:, w_i, :, eo:P:2]
                    for comp in range(2):
                        nc.tensor.matmul(
                            ut_ps[:, comp, eo, :],
                            lhsT,
                            at_sb[:, comp],
                            start=True,
                            stop=True,
                            perf_mode=mybir.MatmulPerfMode.DoubleRow,
                        )
                # PSUM -> SBUF fp8; [p=n%128, comp, even/odd, k]
                ut_sb = utsb_pool.tile([128, 2, 2, 256], dt.float8e4, tag=f"utsb{w_i}")
                if p == 0 and w_i == 0:
                    # warmup: whole utx drain on ACT (idle then); with step1-y
                    # borrowing the fy buffer there is no WAR to hide anymore,
                    # and DVE stays free for the uty drain
                    nc.scalar.copy(ut_sb, ut_ps)
                else:
                    nc.vector.tensor_copy(ut_sb, ut_ps)
                return ut_sb

            def step2_lt(utx, uty, lt):
                """DFT step 2 for l-chunk lt: fields [l%128, w, pm, oc, k] in PSUM."""
                fx_ps = fps_pool.tile([128, 2, 2, 256], dt.float32, tag="fx")
                fy_ps = fps_pool.tile([128, 2, 2, 256], dt.float32, tag="fy")
                for pm in range(2):
                    for oc in range(2):
                        for comp in range(2):
                            lhsT = b_sb[:, lt, pm * 2 + oc, comp]
                            nc.tensor.matmul(
                                fx_ps[:, pm, oc, :], lhsT, utx[:, comp],
                                start=(comp == 0), stop=(comp == 1),
                                perf_mode=mybir.MatmulPerfMode.DoubleRow,
                            )
                            nc.tensor.matmul(
                                fy_ps[:, pm, oc, :], lhsT, uty[:, comp],
                                start=(comp == 0), stop=(comp == 1),
                                perf_mode=mybir.MatmulPerfMode.DoubleRow,
                            )
                return fx_ps, fy_ps

            def elem_a(p, lt, fx_ps, fy_ps):
                # stage A: drain PSUM -> SBUF bf16, square, |Fx|^2
                cx = elem_pool.tile([128, 2, 2, 256], dt.bfloat16, tag="cx")
                nc.scalar.copy(cx, fx_ps)
                cy = elem_pool.tile([128, 2, 2, 256], dt.bfloat16, tag="cy")
                if p == PAIRS_PER_CORE - 1 and lt == 0:
                    # last pair, chunk 0: DVE is out of ut-drain work (no
                    # pair 6) and idles ~1.5us exactly here -- this cy drain
                    # comes off the ACT stream for free
                    nc.vector.tensor_copy(cy, fy_ps)
                else:
                    nc.scalar.copy(cy, fy_ps)
                sq = elem_pool.tile([128, 2, 2, 256], dt.bfloat16, tag="sq")
                if p == PAIRS_PER_CORE - 1 and lt == 1:
                    # drain phase: keep the serial sq->a->rw chain on DVE to
                    # skip two cross-engine semaphore hops
                    nc.vector.tensor_mul(sq, cx, cx)
                else:
                    nc.gpsimd.tensor_mul(sq, cx, cx)
                a_sb = elem_pool.tile([128, 2, 256], dt.bfloat16, tag="a")
                nc.vector.tensor_add(a_sb, sq[:, :, 0], sq[:, :, 1])
                return cx, cy, a_sb

            def elem_b(state):
                # stage B: rw ~= 1/a via the bf16 bit-hack (magic - bits,
                # ~3% rel err -- the loss tolerates ~100x more), ry = Fy * rw
                p, lt, cx, cy, a_sb = state
                rw = elem_pool.tile([128, 2, 256], dt.bfloat16, tag="rw")
                nc.vector.tensor_scalar(
                    rw.bitcast(dt.uint16), a_sb.bitcast(dt.uint16),
                    -1.0, float(0x7EF0), op0=ALU.mult, op1=ALU.add,
                )
                ry = elem_pool.tile([128, 2, 2, 256], dt.bfloat16, tag="ry")
                if p == PAIRS_PER_CORE - 1 and lt == 1:
                    # drain phase: DVE is idle here and the two POOL TTs would
                    # serialize on the tail critical path
                    nc.vector.tensor_mul(ry[:, :, 0], cy[:, :, 0], rw)
                    nc.gpsimd.tensor_mul(ry[:, :, 1], cy[:, :, 1], rw)
                else:
                    nc.gpsimd.tensor_mul(ry[:, :, 0], cy[:, :, 0], rw)
                    nc.gpsimd.tensor_mul(ry[:, :, 1], cy[:, :, 1], rw)
                return p, lt, cx, cy, ry

            pair_d = {}

            def elem_c(state):
                # stage C: Gram-accumulate dots for this chunk; both l-chunks
                # of a pair share one PSUM bank, extracted once per pair.
                # Tail shortcut: the LAST pair stops its Gram after l-chunk 0
                # (extraction overlaps chunk 1) and computes chunk 1's dots as
                # two direct STT reductions -- no PSUM Gram / extraction on
                # the critical tail path.
                p, lt, cx, cy, ry = state
                c0 = p * 8
                _elem_c_body(p, lt, cx, cy, ry, c0)

            def _elem_c_body(p, lt, cx, cy, ry, c0):
                last = p == PAIRS_PER_CORE - 1
                if last and lt == 1:
                    # tail shortcut, hybrid: S1 as a PE mini-Gram (8 matmuls,
                    # can start on the first ry half) in PARALLEL with S2 as
                    # one DVE STT reduction (TensorScalarPtr is not a legal
                    # Pool opcode on HW, so only one STT rides DVE).
                    # reuse the d-ring (same shape) for the S1 mini-Gram
                    d2 = dot_pool.tile([128, 2, 128], dt.float32, tag="d")
                    order = (0, 1, 4, 5, 2, 3, 6, 7)
                    for i, c in enumerate(order):
                        a_i, b_i, h = c // 4, (c // 2) % 2, c % 2
                        nc.tensor.matmul(
                            d2[:, 0, :],
                            cx[:, a_i, b_i, h * 128 : (h + 1) * 128],
                            ry[:, a_i, b_i, h * 128 : (h + 1) * 128],
                            start=(i == 0), stop=(i == 7),
                        )
                    tr2 = elem_pool.tile([128, 2, 2, 256], dt.bfloat16, tag="tr2")
                    col2 = elem_pool.tile([128, 1], dt.float32, tag="col2")
                    nc.vector.scalar_tensor_tensor(
                        tr2, cy, 1.0, ry, op0=ALU.mult, op1=ALU.mult,
                        accum_out=col2,
                    )
                    dsb2 = elem_pool.tile([128, 128], dt.float32, tag="dsb2")
                    nc.scalar.copy(dsb2, d2[:, 0, :])  # ACT is idle on the tail
                    nc.sync.dma_start(out2_d[0], dsb2)
                    nc.scalar.dma_start(out2_d[1, :, 0:1], col2)
                    return
                if lt == 0:
                    d_tile = dot_pool.tile([128, 2, 128], dt.float32, tag="d")
                    pair_d[p] = d_tile
                d = pair_d[p]
                # S1 group first (oc=0 blocks lead: their ry half lands first).
                order = (0, 1, 4, 5, 2, 3, 6, 7)
                for which in range(2):
                    op = cx if which == 0 else cy
                    for i, c in enumerate(order):
                        a_i, b_i, h = c // 4, (c // 2) % 2, c % 2
                        opc = op[:, a_i, b_i, h * 128 : (h + 1) * 128]
                        ryc = ry[:, a_i, b_i, h * 128 : (h + 1) * 128]
                        st = (lt == 0 and i == 0)
                        sp = (lt == 1 or last) and i == 7
                        nc.tensor.matmul(d[:, which], opc, ryc, start=st, stop=sp)
                if lt == 1 or last:
                    # ship the raw Gram; the host extracts the two diagonals.
                    # The last pair's (chunk-0-only) Gram ships mid-stream,
                    # on ACT so it never blocks the tail chain's DVE ops.
                    dsb = elem_pool.tile([128, 2, 128], dt.float32, tag="dsb")
                    if p >= PAIRS_PER_CORE - 2:
                        # late pairs: ACT has tail slack and DVE is on the
                        # critical sq->a->rw chain
                        nc.scalar.copy(dsb, d)
                    else:
                        nc.vector.tensor_copy(dsb, d)
                    nc.sync.dma_start(out_d[p], dsb)
                    del pair_d[p]

            def chunk_a(p, utx, uty, lt):
                fx_ps, fy_ps = step2_lt(utx, uty, lt)
                return (p, lt) + elem_a(p, lt, fx_ps, fy_ps)

            # 3-deep software pipeline over the 12 (pair, lt) chunks:
            # A(c) | B(c-1) | C(c-2), with step1 of the next pair interleaved.
            # advance takes a THUNK so the older chunks' dots/extractions are
            # emitted (and queued) BEFORE the new chunk's step2/drains --
            # frees dot PSUM banks earlier and keeps the tail DVE chain clean.
            stage_b = stage_c = None

            def advance(state_a):
                nonlocal stage_b, stage_c
                if stage_c is not None:
                    elem_c(stage_c)
                    stage_c = None
                if stage_b is not None:
                    stage_c = elem_b(stage_b)
                stage_b = state_a

            pending = None
            for p in range(PAIRS_PER_CORE):
                # chunk 0 of the pending pair is emitted BEFORE this pair's
                # step1 so its step2 matmuls win the PE priority tie against
                # the (slack-rich) step1 -- pulls every pair's first drain in
                if pending is not None:
                    advance(chunk_a(pending[0], pending[1], pending[2], 0))
                utx = step1(p, 0)
                if pending is not None:
                    advance(chunk_a(pending[0], pending[1], pending[2], 1))
                uty = step1(p, 1)
                pending = (p, utx, uty)
            q, ux, uy = pending
            advance(chunk_a(q, ux, uy, 0))
            advance(chunk_a(q, ux, uy, 1))
            advance(None)
            if stage_c is not None:
                elem_c(stage_c)

    nc.finalize()  # Bacc: runs wait-splitting (1-wait/inst HW limit), reg alloc
    _NC = nc
    return nc


# ---------------------------------------------------------------- host side
def _host_corrections(x, y, A_full):
    """Exact (float64) k=0-row and l=0-col sums of cr/a and b/a for one pair."""
    x = x.astype(np.float64)
    y = y.astype(np.float64)
    # l=0 column: F[k,0] = A @ row-sums (sum over n)
    Fx0 = A_full @ x.sum(axis=1)
    Fy0 = A_full @ y.sum(axis=1)
    a0 = np.abs(Fx0) ** 2
    s1c = ((np.conj(Fx0) * Fy0).real / a0).sum()
    s2c = (np.abs(Fy0) ** 2 / a0).sum()
    # k=0 row, l=0..255: F[0,l] = A[:256] @ col-sums (sum over m)
    Fx1 = A_full[:256] @ x.sum(axis=0)
    Fy1 = A_full[:256] @ y.sum(axis=0)
    a1 = np.abs(Fx1) ** 2
    s1r = ((np.conj(Fx1) * Fy1).real / a1).sum()
    s2r = (np.abs(Fy1) ** 2 / a1).sum()
    return s1c, s2c, s1r, s2r


def kernel(recon, target):
    atc, bstk, ident, A_full = _consts()
    f8 = mybir.dt.np(dt.float8e4)
    xs = target.reshape(48, 128, 2, P).astype(f8)  # x = target; m -> (i, j)
    ys = recon.reshape(48, 128, 2, P).astype(f8)  # y = recon
    xy = np.ascontiguousarray(np.stack([xs, ys], axis=2))  # [48, 128, 2, 2, 256]

    nc = _build()
    in_maps = [
        {
            "xy": xy[c * PAIRS_PER_CORE : (c + 1) * PAIRS_PER_CORE],
            "atc": atc,
            "bstk": bstk,
        }
        for c in range(NCORES)
    ]
    res = None
    for attempt in range(3):
        try:
            res = run_bass_kernel_spmd(nc, in_maps, core_ids=list(range(NCORES)))
            break
        except Exception:
            if attempt == 2:
                raise
            import time as _time

            _time.sleep(2.0)

    NN = float(N) * float(N)
    loss = 0.0
    for c in range(NCORES):
        grams = res.results[c]["out"].astype(np.float64)  # [6, 128, 2, 128]
        cols2 = res.results[c]["out2"].astype(np.float64)  # [2, 128, 1]
        kk = np.arange(128)
        for p in range(PAIRS_PER_CORE):
            s1_dev = grams[p, kk, 0, kk]
            s2_dev = grams[p, kk, 1, kk]
            if p == PAIRS_PER_CORE - 1:
                # tail shortcut: last pair's Gram covers chunk 0 only;
                # chunk 1 arrives as a raw S1 mini-Gram + an S2 column
                s1_dev = np.concatenate([s1_dev, cols2[0, kk, kk]])
                s2_dev = np.concatenate([s2_dev, cols2[1, :, 0]])
            pair = c * PAIRS_PER_CORE + p
            b, ch = divmod(pair, 3)
            s1c, s2c, s1r, s2r = _host_corrections(
                np.asarray(target[b, ch]), np.asarray(recon[b, ch]), A_full
            )
            S1 = 2.0 * (s1_dev.sum() - s1r) - s1c
            S2 = 2.0 * (s2_dev.sum() - s2r) - s2c
            v00 = S1 / NN
            E = S2 / NN
            loss += 0.5 * (1.0 - v00 * v00 / E)
    return np.float32(loss)



# revision 72
# speedup vs baseline: 1.0926x; 1.0100x over previous
"""AWLoss (adaptive Wiener filter loss) Trainium2 kernel, 8-core data-parallel.

Math (analytic reduction verified against the reference to ~2e-8 rel err):
  The penalty T (std=1e-4) is numerically 1 everywhere except 0 at the center
  pixel, and the roll puts that center at pre-roll [0,0]. The loss collapses to
      loss = sum_{b,c} 0.5 * (1 - v00^2 / E)
  with, per (b,c) pair (x = target, y = recon, A = 511x256 padded-DFT matrix):
      Fx = A x A^T,  Fy = A y A^T          (full 511x511 spectra)
      a = |Fx|^2, cr = Re(conj(Fx) Fy), b = |Fy|^2
      v00 = sum(cr/a) / N^2,   E = sum(b/a) / N^2       (N = 511)
  The flip-phase factor cancels between numerator and denominator and the
  eps=1e-9 pre-whitening is negligible (|Fx|^2 >~ 0.5 everywhere).

Spectral coverage: the device computes q-sums over rows k = 0..255 ("+"
fields) and rows (511-k) mod 511 ("-" fields, via conjugated DFT stacks
re-using the same Ut), columns l = 0..255. With Hermitian symmetry
  S_full = 2*(S_device - S_row0_half) - S_col0
where the k=0 row (duplicated on device) and l=0 column sums are recomputed
exactly on the host from row/column sums of x and y (tiny 1-D DFTs).

Device pipeline, per (b,c) pair, software-pipelined over 12 (pair, l-chunk)
chunks (all matmuls fp8-e4m3 with DoubleRow pairing, fp32 PSUM accum):
  step1 (PE):  Ut[n,k] = sum_m x[m,n] A[k,m], k = 0..255
  ut copy (DVE): PSUM -> SBUF fp8
  step2 (PE):  F(+-)[l,k] = sum_n B(+-)[n,l] Ut[n,k], per l-chunk of 128
  cx/cy (ACT): F fields PSUM -> SBUF bf16
  sq,ry (POOL), a (DVE): |Fx|^2 and ry = Fy * rw
  rw (DVE):    1/a via the bf16 bit-hack (0x7EF0 - bits, ~3% err; the loss
               term tolerates ~100x more)
  dots (PE):   S1 = sum cx.ry, S2 = sum cy.ry as Gram-matrix accumulations
               over 128x128 chunks; diagonal extracted with a
               scalar_tensor_tensor against the identity (accum_out).
Host: Hermitian corrections, v00/E ratios, final sum in float64.

Toolchain notes: bacc.Bacc + explicit finalize() (walrus allows at most one
sem wait per instruction; Bacc's generate_event_semaphores splits them).
Engine-legality notes (BIR verifier): GPSIMD/Pool cannot touch PSUM (all
PSUM drains must be ACT or DVE), and dma_start cannot source PSUM -- both
constraints shape the drain/dot structure above. Lead-in: pair-0's step1-y
borrows the empty fy PSUM buffer so it needn't WAR-wait on the utx drain
(the fy ring then orders the first fy matmuls after the uty drain).
"""

import os
import sys
from contextlib import ExitStack
from functools import lru_cache

import numpy as np

sys.path.insert(0, "/opt/trn_rl_repo")

import ml_dtypes

import concourse.bacc as bacc
import concourse.mybir as mybir
import concourse.tile as tile
from concourse.bass_utils import run_bass_kernel_spmd

dt = mybir.dt
ALU = mybir.AluOpType

N = 511
OFF = 127
P = 256
NCORES = 8
PAIRS_PER_CORE = 6  # 48 (b,c) pairs / 8 cores


# ---------------------------------------------------------------- constants
@lru_cache(maxsize=1)
def _consts():
    k = np.arange(256)
    m = np.arange(P)
    # step1 rhs AT[m, k] = A[k, m] = exp(-2pi i k (m+OFF) / N), k = 0..255
    theta = 2.0 * np.pi * np.outer(k, (m + OFF)) / N  # [k, m]
    atc = np.stack([np.cos(theta).T, -np.sin(theta).T])  # [2, m, k]
    atc = atc.reshape(2, 128, 2, 256)  # m -> (i, j) DoubleRow pairs
    # step2 lhsT B[n, l] = A[l, n]: Br = cos(2pi l (n+OFF)/N), Bi = -sin
    phi = 2.0 * np.pi * np.outer(m + OFF, np.arange(256)) / N  # [n, l]
    Br = np.cos(phi)
    Bi = -np.sin(phi)
    # stacked lhsT over contraction rows s: s=0,1 multiply Utr, s=2,3 Uti.
    # F+ = (Br + iBi).(Utr + iUti):  F+r = Br.Utr - Bi.Uti ; F+i = Bi.Utr + Br.Uti
    # F- = (Br + iBi).conj(Ut):      F-r = Br.Utr + Bi.Uti ; F-i = Bi.Utr - Br.Uti
    b2 = np.empty((2, 2, 2, 256, 256))  # [pm, oc, comp(of Ut), n, l]
    for pm, sgn in ((0, 1.0), (1, -1.0)):
        b2[pm, 0, 0] = Br
        b2[pm, 1, 0] = Bi
        b2[pm, 0, 1] = -sgn * Bi
        b2[pm, 1, 1] = sgn * Br
    # DoubleRow pairing n = 2i + j: [pm, oc, comp, i, j, l] -> [i, lt, (pm oc), comp, j, l%128]
    # flattened per partition so the load DMA is one contiguous 4 KB run
    # (>=512 B inner run avoids the cost model's 2x small-elem latency mult).
    b2 = b2.reshape(2, 2, 2, 128, 2, 2, 128)  # [pm, oc, comp, i, j, lt, l128]
    bstk2 = np.ascontiguousarray(
        np.transpose(b2, (3, 5, 0, 1, 2, 4, 6)).reshape(128, 4096)
    )
    bf16 = ml_dtypes.bfloat16
    f8 = mybir.dt.np(dt.float8e4)
    # full complex A (float64) for host-side row/col corrections
    kk = np.arange(N)
    A_full = np.exp(-2j * np.pi * np.outer(kk, (m + OFF)) / N)  # [511, 256]
    ident = np.eye(128)
    return atc.astype(f8), bstk2.astype(f8), ident.astype(bf16), A_full


# ---------------------------------------------------------------- bass build
_NC = None


def _build():
    global _NC
    if _NC is not None:
        return _NC
    nc = bacc.Bacc()
    xy_d = nc.dram_tensor("xy", [PAIRS_PER_CORE, 128, 2, 2, P], dt.float8e4, kind="ExternalInput")
    atc_d = nc.dram_tensor("atc", [2, 128, 2, 256], dt.float8e4, kind="ExternalInput")
    bstk_d = nc.dram_tensor("bstk", [128, 4096], dt.float8e4, kind="ExternalInput")
    out_d = nc.dram_tensor("out", [PAIRS_PER_CORE, 128, 2, 128], dt.float32, kind="ExternalOutput")
    # last pair's chunk-1 dots (tail shortcut): slot 0 = raw S1 mini-Gram,
    # slot 1 (column 0) = S2 STT accumulation column
    out2_d = nc.dram_tensor("out2", [2, 128, 128], dt.float32, kind="ExternalOutput")

    with tile.TileContext(nc) as tc:
        with ExitStack() as ctx:
            consts = ctx.enter_context(tc.tile_pool(name="consts", bufs=1))
            # ACT table prime: pay the 1283 ns activation-table load at t~0
            # on the idle ACT engine instead of inside the first real drain.
            prime = consts.tile([128, 1], dt.bfloat16, tag="prime")
            prime2 = consts.tile([128, 1], dt.bfloat16, tag="prime2")
            nc.gpsimd.memset(prime, 0.0)
            nc.scalar.copy(prime2, prime)
            # AT tiles (fp8 DoubleRow): [p=i, comp, j, k] on the ACT hwdge
            # queue so it doesn't serialize behind the pair-0 input DMA.
            at_sb = consts.tile([128, 2, 2, 256], dt.float8e4)
            nc.scalar.dma_start(at_sb, atc_d.rearrange("c p j k -> p c j k"))
            # B tiles (fp8 DoubleRow): [p=i, lt, g=(pm,oc), comp, j, l%128],
            # shipped pre-flattened so the load is ONE contiguous 4 KB/row DMA
            # (1x latency class) on the gpsimd queue.
            b_sb = consts.tile([128, 2, 4, 2, 2, 128], dt.float8e4)
            nc.gpsimd.dma_start(b_sb, bstk_d[:])
            # inputs (fp8 DoubleRow): per-pair tiles [p=i, which, j, n], m = 2i+j
            # one combined x+y DMA per pair (contiguous 1 KB rows)
            xy_tiles = []
            for q in range(PAIRS_PER_CORE):
                xyq = consts.tile([128, 2, 2, P], dt.float8e4, tag=f"xy{q}")
                nc.sync.dma_start(xyq, xy_d[q])
                xy_tiles.append(xyq)

            utps_pool = ctx.enter_context(tc.tile_pool(name="utps", bufs=1, space="PSUM"))
            fps_pool = ctx.enter_context(tc.tile_pool(name="fps", bufs=1, space="PSUM"))
            dot_pool = ctx.enter_context(tc.tile_pool(name="dot", bufs=2, space="PSUM"))
            utsb_pool = ctx.enter_context(tc.tile_pool(name="utsb", bufs=8))
            elem_pool = ctx.enter_context(tc.tile_pool(name="elem", bufs=12))

            def step1(p, w_i):
                """DFT step 1 for pair p, input w_i (0=x,1=y): Ut[n, k0..255]."""
                if p == 0 and w_i == 1:
                    # lead-in: borrow the (still empty) fy PSUM buffer so
                    # step1-y doesn't WAR-wait on the utx drain; the fy ring
                    # naturally orders the first fy matmuls after this drain.
                    ut_ps = fps_pool.tile([128, 2, 2, 256], dt.float32, tag="fy")
                else:
                    ut_ps = utps_pool.tile([128, 2, 2, 256], dt.float32, tag="utps")
                for eo in range(2):
                    lhsT = xy_tiles[p][:, w_i, :, eo:P:2]
                    for comp in range(2):
                        nc.tensor.matmul(
                            ut_ps[:, comp, eo, :],
                            lhsT,
                            at_sb[:, comp],
                            start=True,
                            stop=True,
                            perf_mode=mybir.MatmulPerfMode.DoubleRow,
                        )
                # PSUM -> SBUF fp8; [p=n%128, comp, even/odd, k]
                ut_sb = utsb_pool.tile([128, 2, 2, 256], dt.float8e4, tag=f"utsb{w_i}")
                if p == 0 and w_i == 0:
                    # warmup: whole utx drain on ACT (idle then); with step1-y
                    # borrowing the fy buffer there is no WAR to hide anymore,
                    # and DVE stays free for the uty drain
                    nc.scalar.copy(ut_sb, ut_ps)
                else:
                    nc.vector.tensor_copy(ut_sb, ut_ps)
                return ut_sb

            def step2_lt(utx, uty, lt):
                """DFT step 2 for l-chunk lt: fields [l%128, w, pm, oc, k] in PSUM."""
                fx_ps = fps_pool.tile([128, 2, 2, 256], dt.float32, tag="fx")
                fy_ps = fps_pool.tile([128, 2, 2, 256], dt.float32, tag="fy")
                for pm in range(2):
                    for oc in range(2):
                        for comp in range(2):
                            lhsT = b_sb[:, lt, pm * 2 + oc, comp]
                            nc.tensor.matmul(
                                fx_ps[:, pm, oc, :], lhsT, utx[:, comp],
                                start=(comp == 0), stop=(comp == 1),
                                perf_mode=mybir.MatmulPerfMode.DoubleRow,
                            )
                            nc.tensor.matmul(
                                fy_ps[:, pm, oc, :], lhsT, uty[:, comp],
                                start=(comp == 0), stop=(comp == 1),
                                perf_mode=mybir.MatmulPerfMode.DoubleRow,
                            )
                return fx_ps, fy_ps

            def elem_a(p, lt, fx_ps, fy_ps):
                # stage A: drain PSUM -> SBUF bf16, square, |Fx|^2
                cx = elem_pool.tile([128, 2, 2, 256], dt.bfloat16, tag="cx")
                nc.scalar.copy(cx, fx_ps)
                cy = elem_pool.tile([128, 2, 2, 256], dt.bfloat16, tag="cy")
                if p == PAIRS_PER_CORE - 1 and lt == 0:
                    # last pair, chunk 0: DVE is out of ut-drain work (no
                    # pair 6) and idles ~1.5us exactly here -- this cy drain
                    # comes off the ACT stream for free
                    nc.vector.tensor_copy(cy, fy_ps)
                else:
                    nc.scalar.copy(cy, fy_ps)
                sq = elem_pool.tile([128, 2, 2, 256], dt.bfloat16, tag="sq")
                if p == PAIRS_PER_CORE - 1 and lt == 1:
                    # drain phase: keep the serial sq->a->rw chain on DVE to
                    # skip two cross-engine semaphore hops
                    nc.vector.tensor_mul(sq, cx, cx)
                else:
                    nc.gpsimd.tensor_mul(sq, cx, cx)
                a_sb = elem_pool.tile([128, 2, 256], dt.bfloat16, tag="a")
                nc.vector.tensor_add(a_sb, sq[:, :, 0], sq[:, :, 1])
                return cx, cy, a_sb

            def elem_b(state):
                # stage B: rw ~= 1/a via the bf16 bit-hack (magic - bits,
                # ~3% rel err -- the loss tolerates ~100x more), ry = Fy * rw
                p, lt, cx, cy, a_sb = state
                rw = elem_pool.tile([128, 2, 256], dt.bfloat16, tag="rw")
                nc.vector.tensor_scalar(
                    rw.bitcast(dt.uint16), a_sb.bitcast(dt.uint16),
                    -1.0, float(0x7EF0), op0=ALU.mult, op1=ALU.add,
                )
                ry = elem_pool.tile([128, 2, 2, 256], dt.bfloat16, tag="ry")
                if p == PAIRS_PER_CORE - 1 and lt == 1:
                    # drain phase: DVE is idle here and the two POOL TTs would
                    # serialize on the tail critical path
                    nc.vector.tensor_mul(ry[:, :, 0], cy[:, :, 0], rw)
                    nc.gpsimd.tensor_mul(ry[:, :, 1], cy[:, :, 1], rw)
                else:
                    nc.gpsimd.tensor_mul(ry[:, :, 0], cy[:, :, 0], rw)
                    nc.gpsimd.tensor_mul(ry[:, :, 1], cy[:, :, 1], rw)
                return p, lt, cx, cy, ry

            pair_d = {}

            def elem_c(state):
                # stage C: Gram-accumulate dots for this chunk; both l-chunks
                # of a pair share one PSUM bank, extracted once per pair.
                # Tail shortcut: the LAST pair stops its Gram after l-chunk 0
                # (extraction overlaps chunk 1) and computes chunk 1's dots as
                # two direct STT reductions -- no PSUM Gram / extraction on
                # the critical tail path.
                p, lt, cx, cy, ry = state
                c0 = p * 8
                _elem_c_body(p, lt, cx, cy, ry, c0)

            def _elem_c_body(p, lt, cx, cy, ry, c0):
                last = p == PAIRS_PER_CORE - 1
                if last and lt == 1:
                    # tail shortcut, hybrid: S1 as a PE mini-Gram (8 matmuls,
                    # can start on the first ry half) in PARALLEL with S2 as
                    # one DVE STT reduction (TensorScalarPtr is not a legal
                    # Pool opcode on HW, so only one STT rides DVE).
                    # reuse the d-ring (same shape) for the S1 mini-Gram
                    d2 = dot_pool.tile([128, 2, 128], dt.float32, tag="d")
                    order = (0, 1, 4, 5, 2, 3, 6, 7)
                    for i, c in enumerate(order):
                        a_i, b_i, h = c // 4, (c // 2) % 2, c % 2
                        nc.tensor.matmul(
                            d2[:, 0, :],
                            cx[:, a_i, b_i, h * 128 : (h + 1) * 128],
                            ry[:, a_i, b_i, h * 128 : (h + 1) * 128],
                            start=(i == 0), stop=(i == 7),
                        )
                    tr2 = elem_pool.tile([128, 2, 2, 256], dt.bfloat16, tag="tr2")
                    col2 = elem_pool.tile([128, 1], dt.float32, tag="col2")
                    nc.vector.scalar_tensor_tensor(
                        tr2, cy, 1.0, ry, op0=ALU.mult, op1=ALU.mult,
                        accum_out=col2,
                    )
                    dsb2 = elem_pool.tile([128, 128], dt.float32, tag="dsb2")
                    nc.scalar.copy(dsb2, d2[:, 0, :])  # ACT is idle on the tail
                    nc.sync.dma_start(out2_d[0], dsb2)
                    nc.scalar.dma_start(out2_d[1, :, 0:1], col2)
                    return
                if lt == 0:
                    d_tile = dot_pool.tile([128, 2, 128], dt.float32, tag="d")
                    pair_d[p] = d_tile
                d = pair_d[p]
                # S1 group first (oc=0 blocks lead: their ry half lands first).
                # Gram matmuls get demoted priority: they have ~2 pair-periods
                # of slack (only dsb/DMA consume them) and must not delay the
                # step2 matmuls that feed the critical ACT drain stream.
                save_prio = tc.cur_priority
                tc.cur_priority += 30
                order = (0, 1, 4, 5, 2, 3, 6, 7)
                for which in range(2):
                    op = cx if which == 0 else cy
                    for i, c in enumerate(order):
                        a_i, b_i, h = c // 4, (c // 2) % 2, c % 2
                        opc = op[:, a_i, b_i, h * 128 : (h + 1) * 128]
                        ryc = ry[:, a_i, b_i, h * 128 : (h + 1) * 128]
                        st = (lt == 0 and i == 0)
                        sp = (lt == 1 or last) and i == 7
                        nc.tensor.matmul(d[:, which], opc, ryc, start=st, stop=sp)
                tc.cur_priority = save_prio
                if lt == 1 or last:
                    # ship the raw Gram; the host extracts the two diagonals.
                    # The last pair's (chunk-0-only) Gram ships mid-stream,
                    # on ACT so it never blocks the tail chain's DVE ops.
                    dsb = elem_pool.tile([128, 2, 128], dt.float32, tag="dsb")
                    if p >= PAIRS_PER_CORE - 2:
                        # late pairs: ACT has tail slack and DVE is on the
                        # critical sq->a->rw chain
                        nc.scalar.copy(dsb, d)
                    else:
                        nc.vector.tensor_copy(dsb, d)
                    nc.sync.dma_start(out_d[p], dsb)
                    del pair_d[p]

            def chunk_a(p, utx, uty, lt):
                fx_ps, fy_ps = step2_lt(utx, uty, lt)
                return (p, lt) + elem_a(p, lt, fx_ps, fy_ps)

            # 3-deep software pipeline over the 12 (pair, lt) chunks:
            # A(c) | B(c-1) | C(c-2), with step1 of the next pair interleaved.
            # advance takes a THUNK so the older chunks' dots/extractions are
            # emitted (and queued) BEFORE the new chunk's step2/drains --
            # frees dot PSUM banks earlier and keeps the tail DVE chain clean.
            stage_b = stage_c = None

            def advance(state_a):
                nonlocal stage_b, stage_c
                if stage_c is not None:
                    elem_c(stage_c)
                    stage_c = None
                if stage_b is not None:
                    stage_c = elem_b(stage_b)
                stage_b = state_a

            pending = None
            for p in range(PAIRS_PER_CORE):
                # chunk 0 of the pending pair is emitted BEFORE this pair's
                # step1 so its step2 matmuls win the PE priority tie against
                # the (slack-rich) step1 -- pulls every pair's first drain in
                if pending is not None:
                    advance(chunk_a(pending[0], pending[1], pending[2], 0))
                utx = step1(p, 0)
                if pending is not None:
                    advance(chunk_a(pending[0], pending[1], pending[2], 1))
                uty = step1(p, 1)
                pending = (p, utx, uty)
            q, ux, uy = pending
            advance(chunk_a(q, ux, uy, 0))
            advance(chunk_a(q, ux, uy, 1))
            advance(None)
            if stage_c is not None:
                elem_c(stage_c)

    nc.finalize()  # Bacc: runs wait-splitting (1-wait/inst HW limit), reg alloc
    _NC = nc
    return nc


# ---------------------------------------------------------------- host side
def _host_corrections(x, y, A_full):
    """Exact (float64) k=0-row and l=0-col sums of cr/a and b/a for one pair."""
    x = x.astype(np.float64)
    y = y.astype(np.float64)
    # l=0 column: F[k,0] = A @ row-sums (sum over n)
    Fx0 = A_full @ x.sum(axis=1)
    Fy0 = A_full @ y.sum(axis=1)
    a0 = np.abs(Fx0) ** 2
    s1c = ((np.conj(Fx0) * Fy0).real / a0).sum()
    s2c = (np.abs(Fy0) ** 2 / a0).sum()
    # k=0 row, l=0..255: F[0,l] = A[:256] @ col-sums (sum over m)
    Fx1 = A_full[:256] @ x.sum(axis=0)
    Fy1 = A_full[:256] @ y.sum(axis=0)
    a1 = np.abs(Fx1) ** 2
    s1r = ((np.conj(Fx1) * Fy1).real / a1).sum()
    s2r = (np.abs(Fy1) ** 2 / a1).sum()
    return s1c, s2c, s1r, s2r


def kernel(recon, target):
    atc, bstk, ident, A_full = _consts()
    f8 = mybir.dt.np(dt.float8e4)
    xs = target.reshape(48, 128, 2, P).astype(f8)  # x = target; m -> (i, j)
    ys = recon.reshape(48, 128, 2, P).astype(f8)  # y = recon
    xy = np.ascontiguousarray(np.stack([xs, ys], axis=2))  # [48, 128, 2, 2, 256]

    nc = _build()
    in_maps = [
        {
            "xy": xy[c * PAIRS_PER_CORE : (c + 1) * PAIRS_PER_CORE],
            "atc": atc,
            "bstk": bstk,
        }
        for c in range(NCORES)
    ]
    res = None
    for attempt in range(3):
        try:
            res = run_bass_kernel_spmd(nc, in_maps, core_ids=list(range(NCORES)))
            break
        except Exception:
            if attempt == 2:
                raise
            import time as _time

            _time.sleep(2.0)

    NN = float(N) * float(N)
    loss = 0.0
    for c in range(NCORES):
        grams = res.results[c]["out"].astype(np.float64)  # [6, 128, 2, 128]
        cols2 = res.results[c]["out2"].astype(np.float64)  # [2, 128, 1]
        kk = np.arange(128)
        for p in range(PAIRS_PER_CORE):
            s1_dev = grams[p, kk, 0, kk]
            s2_dev = grams[p, kk, 1, kk]
            if p == PAIRS_PER_CORE - 1:
                # tail shortcut: last pair's Gram covers chunk 0 only;
                # chunk 1 arrives as a raw S1 mini-Gram + an S2 column
                s1_dev = np.concatenate([s1_dev, cols2[0, kk, kk]])
                s2_dev = np.concatenate([s2_dev, cols2[1, :, 0]])
            pair = c * PAIRS_PER_CORE + p
            b, ch = divmod(pair, 3)
            s1c, s2c, s1r, s2r = _host_corrections(
                np.asarray(target[b, ch]), np.asarray(recon[b, ch]), A_full
            )
            S1 = 2.0 * (s1_dev.sum() - s1r) - s1c
            S2 = 2.0 * (s2_dev.sum() - s2r) - s2c
            v00 = S1 / NN
            E = S2 / NN
            loss += 0.5 * (1.0 - v00 * v00 / E)
    return np.float32(loss)

